# revision 2
# baseline (speedup 1.0000x reference)
"""Gemma3 decoder layer on 8 Trainium2 NeuronCores (Bass/Tile), v2.

Sharding (per core c, kv-group g=c//2):
  - S1: in_ln on own 256 tokens; QKV for own tokens x ALL heads with
    fp8-split3 DoubleRow matmuls (w ~ w_hi+w_lo, a ~ a_hi+a_lo fp8e4;
    compute w_hi@a_hi + w_lo@a_hi + w_hi@a_lo = 1.33x PE throughput at
    better-than-bf16 accuracy); AllToAll -> core c owns Q head c and
    KV head g for the full sequence.
  - S2: per 128-token tile: q/k rmsnorm + rope (tables precomputed on
    host with (1+w) folded), transposes into QT/KT; V direct.
  - S3: sliding-window attention (512) per 512-query block; attn^T is
    split to fp8 and immediately consumed by ROW-parallel wo partial
    matmuls (wo sharded by attn-dim rows = my head); partials for all
    2048 tokens -> ReduceScatter lands my 256 tokens summed.
  - S4: post_attn norm + residual + pre_ff norm; h2^T split to fp8
    hi/lo, AllGather.
  - S5: MLP column/row sharded (1280 inter per core), all matmuls
    fp8-split3 DoubleRow; partial down outputs -> ReduceScatter.
  - S6: post_ff norm + residual.
rmsnorm 1/sqrt via Sqrt activation + DVE reciprocal (one act table per
phase; Ln/Exp thrash removed). Norms/softmax/residual in fp32.
"""
import sys

if "/opt/trn_rl_repo" not in sys.path:
    sys.path.insert(0, "/opt/trn_rl_repo")

import numpy as np
import ml_dtypes

import concourse.bass as bass
import concourse.mybir as mybir
import concourse.tile as tile
from concourse import bacc
from concourse.bass_utils import run_bass_kernel_spmd
from concourse.masks import make_identity

dt = mybir.dt
AF = mybir.ActivationFunctionType
ALU = mybir.AluOpType
BF = dt.bfloat16
F8 = dt.float8e4
F32 = dt.float32
DR = mybir.MatmulPerfMode.DoubleRow

HID, NH, NKV, HD, INTER = 2560, 8, 4, 256, 10240
WIN, EPS, BASE = 512, 1e-6, 10000.0
S = 2048
NC_ = 8
TS = S // NC_              # 256 tokens per core
KH = HID // 128            # 20 hidden-dim chunks
MI = INTER // NC_ // 128   # 10 inter m-tiles per core
HALF = HD // 2
QC = NH * HD + 2 * NKV * HD  # 4096 qkv columns
WS = 64.0                  # fp8 weight scale
IWS = 1.0 / WS


def _bcast_row(nc, sbuf_tile, dram_t, width):
    a = dram_t.ap()
    nc.sync.dma_start(sbuf_tile[:], bass.AP(
        tensor=a.tensor, offset=a.offset, ap=[[0, 128], [1, width]]))


def _swap_ap(src):
    """Read [128, 256] AP with free-dim halves swapped (as [128,2,128])."""
    return bass.AP(tensor=src.tensor, offset=src.offset + HALF,
                   ap=[list(src.ap[0]), [-HALF, 2], [1, HALF]])


def build_nc(sim=False):
    nc = bacc.Bacc("TRN2", target_bir_lowering=False, debug=False,
                   enable_asserts=True, num_devices=1 if sim else NC_)

    def _coll(kind, op, ins, outs):
        if not sim:
            nc.gpsimd.collective_compute(kind, op, replica_groups=rg,
                                         ins=ins, outs=outs)
            return
        i_ap, o_ap = ins[0], outs[0]
        if kind == "AllGather":
            n = i_ap.shape[0]
            for r in range(NC_):
                nc.sync.dma_start(o_ap[r * n:(r + 1) * n], i_ap)
        elif kind == "AllToAll":
            nc.sync.dma_start(o_ap, i_ap)
        elif kind == "ReduceScatter":
            n = o_ap.shape[0]
            nc.sync.dma_start(o_ap, i_ap[0:n])

    x_shard = nc.dram_tensor("x_shard", [TS, HID], F32, kind="ExternalInput")
    wqkv_hi = nc.dram_tensor("wqkv_hi", [HID, QC], F8, kind="ExternalInput")
    wqkv_lo = nc.dram_tensor("wqkv_lo", [HID, QC], F8, kind="ExternalInput")
    wo_hi = nc.dram_tensor("wo_hi", [HD, HID], F8, kind="ExternalInput")
    wo_lo = nc.dram_tensor("wo_lo", [HD, HID], F8, kind="ExternalInput")
    wg_hi = nc.dram_tensor("wg_hi", [HID, INTER // NC_], F8, kind="ExternalInput")
    wg_lo = nc.dram_tensor("wg_lo", [MI, 128, KH, 128], F8, kind="ExternalInput")
    wu_hi = nc.dram_tensor("wu_hi", [MI, 128, KH, 128], F8, kind="ExternalInput")
    wu_lo = nc.dram_tensor("wu_lo", [MI, 128, KH, 128], F8, kind="ExternalInput")
    wd_hi = nc.dram_tensor("wd_hi", [INTER // NC_, HID], F8, kind="ExternalInput")
    wd_lo = nc.dram_tensor("wd_lo", [INTER // NC_, HID], F8, kind="ExternalInput")
    w1_in = nc.dram_tensor("w1_in", [HID], BF, kind="ExternalInput")
    w1_pa = nc.dram_tensor("w1_pa", [HID], BF, kind="ExternalInput")
    w1_pf = nc.dram_tensor("w1_pf", [HID], BF, kind="ExternalInput")
    w1_po = nc.dram_tensor("w1_po", [HID], F32, kind="ExternalInput")
    cqw = nc.dram_tensor("cqw", [S, HD], BF, kind="ExternalInput")
    sqw = nc.dram_tensor("sqw", [S, HD], BF, kind="ExternalInput")
    ckw = nc.dram_tensor("ckw", [S, HD], BF, kind="ExternalInput")
    skw = nc.dram_tensor("skw", [S, HD], BF, kind="ExternalInput")
    out_shard = nc.dram_tensor("out_shard", [TS, HID], F32, kind="ExternalOutput")

    rg = [list(range(NC_))]
    stages = {}
    nc._stage_ids = stages

    def mark(name):
        stages[name] = nc.next_id()

    with tile.TileContext(nc) as tc:
        with (
            tc.tile_pool(name="dram", bufs=1, space="DRAM") as dram,
            tc.tile_pool(name="glob", bufs=1) as glob,
            tc.tile_pool(name="nrm", bufs=8) as nrm,
            tc.tile_pool(name="psP", bufs=1, space="PSUM") as psP,
        ):
            # DRAM scratch
            a2a1_in = dram.tile([S, 3 * HD], BF)
            a2a1_out = dram.tile([S, 3 * HD], BF)
            rs1a_in = dram.tile([S // 2, HID], BF)
            rs1a_out = dram.tile([TS // 2, HID], BF)
            rs1b_in = dram.tile([S // 2, HID], BF)
            rs1b_out = dram.tile([TS // 2, HID], BF)
            h2T_in = dram.tile([HID, 2 * TS], F8)
            h2T_full = dram.tile([NC_ * HID, 2 * TS], F8,
                                 addr_space="Local" if sim else "Shared")
            rs2_in = dram.tile([S, HID], BF)
            rs2_out = dram.tile([TS, HID], BF)
            x2_spill = dram.tile([TS, HID], F32)

            ident = glob.tile([128, 128], BF)
            make_identity(nc, ident[:])
            eps_t = glob.tile([128, 1], F32)
            nc.vector.memset(eps_t[:], EPS)

            def rmsnorm_rinv(src_ap, d, name):
                """rinv[p,1] = 1/sqrt(mean(src^2)+EPS): Act Square-accum,
                Act Sqrt(scale=1/d, bias=eps), DVE reciprocal."""
                scr = nrm.tile([128, d], BF, tag=f"nsc{d}", name=f"{name}_sc",
                               bufs=1 if d > 512 else 2)
                ms = nrm.tile([128, 1], F32, tag="nms", name=f"{name}_ms")
                nc.scalar.activation(scr[:], src_ap, AF.Square,
                                     accum_out=ms[:])
                sq = nrm.tile([128, 1], F32, tag="nsq", name=f"{name}_sq")
                nc.scalar.activation(sq[:], ms[:], AF.Sqrt, bias=eps_t[:],
                                     scale=1.0 / d)
                rinv = nrm.tile([128, 1], F32, tag="nrv", name=f"{name}_rv")
                nc.vector.reciprocal(rinv[:], sq[:])
                return rinv

            wgpre_cm = tc.tile_pool(name="wgpre", bufs=1)
            wgpre = wgpre_cm.__enter__()
            wg_pre_hi = wgpre.tile([128, KH, INTER // NC_], F8, name="wg_pre_hi")

            with tc.tile_pool(name="xpool", bufs=1) as xpool:
                x_sb = [xpool.tile([128, HID], F32, name=f"xt{t}") for t in range(2)]

                mark('S1')
                # ============ S1: in_ln + transpose/split + QKV + A2A1 ========
                with tc.tile_pool(name="s1", bufs=2) as s1:
                    w1_in_b = s1.tile([128, HID], BF, bufs=1)
                    _bcast_row(nc, w1_in_b, w1_in, HID)
                    hT_hi = s1.tile([128, KH, TS], F8, bufs=1, name="hT_hi")
                    hT_lo = s1.tile([128, KH, TS], F8, bufs=1, name="hT_lo")
                    qkv16 = [s1.tile([128, QC], BF, bufs=1, name=f"qkv16_{t}")
                             for t in range(2)]
                    h16s = []
                    for t in range(2):
                        nc.sync.dma_start(x_sb[t][:],
                                          x_shard.ap()[t * 128:(t + 1) * 128, :])
                        rinv = rmsnorm_rinv(x_sb[t][:], HID, f"inln{t}")
                        h16 = s1.tile([128, HID], BF, tag="h16", name=f"h16_{t}",
                                      bufs=2)
                        for cch in range(5):
                            sl = slice(cch * 512, (cch + 1) * 512)
                            nc.vector.scalar_tensor_tensor(h16[:, sl], x_sb[t][:, sl],
                                                           rinv[:], w1_in_b[:, sl],
                                                           op0=ALU.mult, op1=ALU.mult)
                        h16s.append(h16)
                    for k in range(KH):
                        ptr = psP.tile([128, TS], BF, tag="tr", bufs=2,
                                       name=f"s1tr{k}")
                        for t in range(2):
                            nc.tensor.transpose(
                                ptr[:, t * 128:(t + 1) * 128],
                                h16s[t][:, k * 128:(k + 1) * 128], ident[:])
                        nc.vector.tensor_copy(hT_hi[:, k, :], ptr[:])
                        nc.vector.tensor_sub(hT_lo[:, k, :], ptr[:], hT_hi[:, k, :])
                    # QKV: full heads for own tokens, fp8 split3
                    for n in range(QC // 512):
                        wn_hi = s1.tile([128, KH, 512], F8, tag="wqn",
                                        name=f"wqh{n}", bufs=3)
                        wn_lo = s1.tile([128, KH, 512], F8, tag="wqnl",
                                        name=f"wql{n}", bufs=3)
                        nc.sync.dma_start(
                            wn_hi[:], wqkv_hi.ap()[:, n * 512:(n + 1) * 512]
                            .rearrange("(k p) n -> p k n", p=128))
                        nc.sync.dma_start(
                            wn_lo[:], wqkv_lo.ap()[:, n * 512:(n + 1) * 512]
                            .rearrange("(k p) n -> p k n", p=128))
                        for t in range(2):
                            pq = psP.tile([128, 512], F32, tag="mm", bufs=6,
                                          name=f"pq{n}_{t}")
                            tsl = slice(t * 128, (t + 1) * 128)
                            for kp in range(KH // 2):
                                ks = slice(2 * kp, 2 * kp + 2)
                                st = (kp == 0)
                                sp = (kp == KH // 2 - 1)
                                nc.tensor.matmul(pq[:], hT_hi[:, ks, tsl],
                                                 wn_hi[:, ks, :], start=st,
                                                 stop=False, perf_mode=DR)
                                nc.tensor.matmul(pq[:], hT_lo[:, ks, tsl],
                                                 wn_hi[:, ks, :], start=False,
                                                 stop=False, perf_mode=DR)
                                nc.tensor.matmul(pq[:], hT_hi[:, ks, tsl],
                                                 wn_lo[:, ks, :], start=False,
                                                 stop=sp, perf_mode=DR)
                            nc.scalar.activation(qkv16[t][:, n * 512:(n + 1) * 512],
                                                 pq[:], AF.Copy, scale=IWS)
                    for d in range(NC_):
                        g = d // 2
                        for t in range(2):
                            rs = slice(TS * d + 128 * t, TS * d + 128 * (t + 1))
                            nc.sync.dma_start(a2a1_in[rs, 0:HD],
                                              qkv16[t][:, HD * d:HD * (d + 1)])
                            nc.sync.dma_start(
                                a2a1_in[rs, HD:2 * HD],
                                qkv16[t][:, NH * HD + HD * g:NH * HD + HD * (g + 1)])
                            nc.sync.dma_start(
                                a2a1_in[rs, 2 * HD:3 * HD],
                                qkv16[t][:, (NH + NKV) * HD + HD * g:
                                          (NH + NKV) * HD + HD * (g + 1)])
                    _coll("AllToAll", ALU.bypass, [a2a1_in[:]], [a2a1_out[:]])
                    nc.sync.dma_start(wg_pre_hi[:], wg_hi.ap()
                                      .rearrange("(k p) n -> p k n", p=128))

                mark('S2')
                # ============ S2/S3: attention + fused row-parallel wo ========
                with tc.tile_pool(name="attp", bufs=1) as attp:
                    ropes = attp.tile([128, 4, S // 128, HD], BF, name="ropes")
                    for ti, tab in enumerate((cqw, sqw, ckw, skw)):
                        nc.sync.dma_start(
                            ropes[:, ti, :, :],
                            tab.ap().rearrange("(t p) d -> p t d", p=128))

                    QTm = attp.tile([128, 2, S], BF, name="QTm")
                    KTm = attp.tile([128, 2, S], BF, name="KTm")
                    V = [attp.tile([128, HD + 1], BF, name=f"V{i}")
                         for i in range(S // 128)]
                    for i in range(S // 128):
                        nc.vector.memset(V[i][:, HD:HD + 1], 1.0)
                    masks = attp.tile([128, 8, 512], BF)
                    for i in range(8):
                        delta = 512 - 128 * i
                        mk = masks[:, i, :]
                        nc.gpsimd.memset(mk, 1.0)
                        nc.gpsimd.affine_select(
                            out=mk, in_=mk, compare_op=ALU.is_ge, fill=0.0,
                            base=delta, pattern=[[1, 512]], channel_multiplier=-1)
                        nc.gpsimd.affine_select(
                            out=mk, in_=mk, compare_op=ALU.is_ge, fill=0.0,
                            base=-delta + (WIN - 1), pattern=[[-1, 512]],
                            channel_multiplier=1)
                    wo_sb_hi = attp.tile([128, 2, HID], F8, name="wo_sb_hi")
                    wo_sb_lo = attp.tile([128, 2, HID], F8, name="wo_sb_lo")
                    nc.sync.dma_start(wo_sb_hi[:],
                                      wo_hi.ap().rearrange("(a p) n -> p a n", p=128))
                    nc.sync.dma_start(wo_sb_lo[:],
                                      wo_lo.ap().rearrange("(a p) n -> p a n", p=128))

                    with tc.tile_pool(name="s2", bufs=2) as s2:
                        for tt in range(S // 128):
                            jj, hh = (tt, 0) if tt < 8 else (tt - 8, 1)
                            base = 256 * jj + 128 * hh
                            rows = slice(base, base + 128)
                            qk_t = s2.tile([128, 2 * HD], BF, tag="qkt",
                                           name=f"qkt{tt}", bufs=3)
                            nc.sync.dma_start(qk_t[:], a2a1_out[rows, 0:2 * HD])
                            nc.sync.dma_start(V[tt][:, 0:HD],
                                              a2a1_out[rows, 2 * HD:3 * HD])
                            for (qo, tbi, QKT, nm) in (
                                    (0, 0, QTm, "q"),
                                    (HD, 2, KTm, "k")):
                                src = qk_t[:, qo:qo + HD]
                                rinv = rmsnorm_rinv(src, HD, f"{nm}n{tt}")
                                ct = ropes[:, tbi, tt, :]
                                st_ = ropes[:, tbi + 1, tt, :]
                                t1 = s2.tile([128, HD], BF, tag="t1",
                                             name=f"t1{nm}{tt}", bufs=3)
                                t2 = s2.tile([128, HD], BF, tag="t2",
                                             name=f"t2{nm}{tt}", bufs=3)
                                nc.vector.scalar_tensor_tensor(
                                    t1[:], src, rinv[:], ct,
                                    op0=ALU.mult, op1=ALU.mult)
                                nc.gpsimd.tensor_mul(
                                    t2[:].rearrange("p (a b) -> p a b", a=2),
                                    _swap_ap(src),
                                    st_.rearrange("p (a b) -> p a b", a=2))
                                qr = s2.tile([128, HD], BF, tag="qr",
                                             name=f"qr{nm}{tt}", bufs=3)
                                nc.vector.scalar_tensor_tensor(
                                    qr[:], t2[:], rinv[:], t1[:],
                                    op0=ALU.mult, op1=ALU.add)
                                ptr = psP.tile([128, HD], BF, tag="tr",
                                               bufs=2, name=f"s2t{nm}{tt}")
                                for h in range(2):
                                    nc.tensor.transpose(
                                        ptr[:, h * 128:(h + 1) * 128],
                                        qr[:, h * 128:(h + 1) * 128], ident[:])
                                nc.vector.tensor_copy(
                                    QKT[:, :, 128 * tt:128 * (tt + 1)],
                                    ptr[:].rearrange("p (a b) -> p a b", a=2))

                    mark('S3')
                    with tc.tile_pool(name="s3", bufs=2) as s3:
                        w1_pa_b = s3.tile([128, HID], BF, bufs=1, name="w1pab")
                        w1_pf_b = s3.tile([128, HID], BF, bufs=1, name="w1pfb")
                        _bcast_row(nc, w1_pa_b, w1_pa, HID)
                        _bcast_row(nc, w1_pf_b, w1_pf, HID)
                        h2Ts = s3.tile([128, KH, 2 * TS], F8, bufs=1, name="h2Ts")

                        def s4_half(t, rs_out_t):
                            ao = s3.tile([128, HID], BF, tag="ao", name=f"ao{t}",
                                         bufs=2)
                            nc.sync.dma_start(ao[:], rs_out_t[:])
                            rinv_a = rmsnorm_rinv(ao[:], HID, f"pan{t}")
                            x2 = s3.tile([128, HID], F32, tag="x2",
                                         name=f"x2_{t}", bufs=1)
                            for cch in range(5):
                                sl = slice(cch * 512, (cch + 1) * 512)
                                nc.vector.scalar_tensor_tensor(
                                    x2[:, sl], ao[:, sl], rinv_a[:],
                                    w1_pa_b[:, sl], op0=ALU.mult, op1=ALU.mult)
                                nc.gpsimd.tensor_add(x2[:, sl], x2[:, sl],
                                                      x_sb[t][:, sl])
                            nc.sync.dma_start(
                                x2_spill[t * 128:(t + 1) * 128, :], x2[:])
                            rinv_f = rmsnorm_rinv(x2[:], HID, f"pff{t}")
                            h2 = s3.tile([128, HID], BF, tag="h2", name=f"h2_{t}",
                                         bufs=2)
                            for cch in range(5):
                                sl = slice(cch * 512, (cch + 1) * 512)
                                nc.vector.scalar_tensor_tensor(
                                    h2[:, sl], x2[:, sl], rinv_f[:],
                                    w1_pf_b[:, sl], op0=ALU.mult, op1=ALU.mult)
                            for k in range(KH):
                                ptr = psP.tile([128, 128], BF, tag="tr", bufs=2,
                                               name=f"s4tr{k}_{t}")
                                nc.tensor.transpose(
                                    ptr[:], h2[:, k * 128:(k + 1) * 128],
                                    ident[:])
                                tsl = slice(t * 128, (t + 1) * 128)
                                lsl = slice(TS + t * 128, TS + (t + 1) * 128)
                                nc.vector.tensor_copy(h2Ts[:, k, tsl], ptr[:])
                                nc.vector.tensor_sub(h2Ts[:, k, lsl], ptr[:],
                                                     h2Ts[:, k, tsl])

                        for qb in range(4):
                            q0 = 512 * qb
                            probs = {}
                            for i in range(8):
                                kc = q0 - 512 + 128 * i
                                if kc < 0:
                                    continue
                                psc = psP.tile([128, 512], F32, tag="mm", bufs=6,
                                               name=f"psc{qb}_{i}")
                                for h in range(2):
                                    nc.tensor.matmul(psc[:], KTm[:, h, kc:kc + 128],
                                                     QTm[:, h, q0:q0 + 512],
                                                     start=(h == 0), stop=(h == 1))
                                pr = s3.tile([128, 512], BF, tag="pr",
                                             name=f"pr{qb}_{i}", bufs=10)
                                nc.scalar.activation(pr[:], psc[:], AF.Exp,
                                                     scale=1.0 / 16.0)
                                nc.gpsimd.tensor_mul(pr[:], pr[:], masks[:, i, :])
                                probs[kc] = pr
                            for qs in range(4):
                                qa = q0 + 128 * qs
                                kcs = [kc for kc in range(qa - 512, qa + 128, 128)
                                       if kc >= 0]
                                po = psP.tile([128, HD + 1], F32, tag="mm", bufs=6,
                                              name=f"po{qb}_{qs}")
                                col = qa - q0
                                for j, kc in enumerate(kcs):
                                    nc.tensor.matmul(po[:],
                                                     probs[kc][:, col:col + 128],
                                                     V[kc // 128][:], start=(j == 0),
                                                     stop=(j == len(kcs) - 1))
                                rec = s3.tile([128, 1], F32, tag="rec",
                                              name=f"rec{qb}_{qs}")
                                nc.vector.reciprocal(rec[:], po[:, HD:HD + 1])
                                an = s3.tile([128, HD], BF, tag="an",
                                             name=f"an{qb}_{qs}")
                                nc.vector.tensor_scalar_mul(an[:], po[:, 0:HD],
                                                            rec[:])
                                ptr = psP.tile([128, HD], BF, tag="tr", bufs=2,
                                               name=f"s3tr{qb}{qs}")
                                for h in range(2):
                                    nc.tensor.transpose(
                                        ptr[:, h * 128:(h + 1) * 128],
                                        an[:, h * 128:(h + 1) * 128], ident[:])
                                aT_hi = s3.tile([128, 2, 128], F8, tag="aTh",
                                                name=f"aTh{qb}_{qs}", bufs=3)
                                aT_lo = s3.tile([128, 2, 128], F8, tag="aTl",
                                                name=f"aTl{qb}_{qs}", bufs=3)
                                p3 = ptr[:].rearrange("p (a b) -> p a b", a=2)
                                nc.vector.tensor_copy(aT_hi[:], p3)
                                nc.vector.tensor_sub(aT_lo[:], p3, aT_hi[:])
                                # row-parallel wo partials for this token tile
                                xo = s3.tile([128, HID], BF, tag="xo",
                                             name=f"xo{qb}_{qs}", bufs=3)
                                for n in range(5):
                                    pw = psP.tile([128, 512], F32, tag="mm",
                                                  bufs=6, name=f"pw{qb}{qs}_{n}")
                                    nsl = slice(n * 512, (n + 1) * 512)
                                    nc.tensor.matmul(pw[:], aT_hi[:],
                                                     wo_sb_hi[:, :, nsl],
                                                     start=True, stop=False,
                                                     perf_mode=DR)
                                    nc.tensor.matmul(pw[:], aT_lo[:],
                                                     wo_sb_hi[:, :, nsl],
                                                     start=False, stop=False,
                                                     perf_mode=DR)
                                    nc.tensor.matmul(pw[:], aT_hi[:],
                                                     wo_sb_lo[:, :, nsl],
                                                     start=False, stop=True,
                                                     perf_mode=DR)
                                    if n % 2 == 0:
                                        nc.scalar.activation(xo[:, nsl], pw[:],
                                                             AF.Copy, scale=IWS)
                                    else:
                                        nc.vector.tensor_scalar_mul(
                                            xo[:, nsl], pw[:], IWS)
                                qq = qa // 128
                                if qq < 8:
                                    nc.sync.dma_start(
                                        rs1a_in[128 * qq:128 * (qq + 1), :], xo[:])
                                else:
                                    nc.sync.dma_start(
                                        rs1b_in[128 * (qq - 8):128 * (qq - 7), :],
                                        xo[:])
                            if qb == 1:
                                _coll("ReduceScatter", ALU.add, [rs1a_in[:]],
                                      [rs1a_out[:]])
                                s4_half(0, rs1a_out)
                            elif qb == 3:
                                _coll("ReduceScatter", ALU.add, [rs1b_in[:]],
                                      [rs1b_out[:]])
                                s4_half(1, rs1b_out)
                        nc.sync.dma_start(
                            h2T_in[:].rearrange("(k p) t -> p k t", p=128),
                            h2Ts[:])
                        if not sim:
                            _coll("AllGather", ALU.bypass, [h2T_in[:]],
                                  [h2T_full[:]])

            mark('S5')
            # ================= S5: MLP (fp8 split3) =================
            with tc.tile_pool(name="s5w", bufs=1) as s5w:
                h2T_sb = s5w.tile([128, KH, NC_, 2 * TS], F8, name="h2T_sb")
                actT_hi = s5w.tile([128, MI, S], F8, name="actT_hi")
                actT_lo = s5w.tile([128, MI, S], F8, name="actT_lo")
                with tc.tile_pool(name="s5", bufs=2) as s5:
                    def load_wu(m):
                        d = {}
                        for (sfx, t_) in (("h", wu_hi), ("l", wu_lo),
                                          ("gl", wg_lo)):
                            w_ = s5.tile([128, KH, 128], F8, tag=f"wu{sfx}",
                                         name=f"wu{sfx}{m}", bufs=2)
                            nc.sync.dma_start(w_[:], t_.ap()[m])
                            d[sfx] = w_
                        return d

                    wu_tiles = {0: load_wu(0)}
                    for r in range(NC_):
                        blk = h2T_full[r * HID:(r + 1) * HID]
                        if sim:
                            nc.sync.dma_start(blk, h2T_in[:])
                        nc.sync.dma_start(
                            h2T_sb[:, :, r, :],
                            blk.rearrange("(k p) t -> p k t", p=128))
                        if r == 1:
                            wu_tiles[1] = load_wu(1)
                    for m in range(MI):
                        msl = slice(m * 128, (m + 1) * 128)
                        wu_t = wu_tiles.pop(m)
                        if m + 2 < MI:
                            wu_tiles[m + 2] = load_wu(m + 2)
                        for r in range(0, NC_, 2):
                            pg = psP.tile([128, 512], F32, tag="mm", bufs=6,
                                          name=f"pg{m}_{r}")
                            pu = psP.tile([128, 512], F32, tag="mm", bufs=6,
                                          name=f"pu{m}_{r}")
                            for kp in range(KH // 2):
                                ks = slice(2 * kp, 2 * kp + 2)
                                ah = h2T_sb[:, ks, r:r + 2, 0:TS]
                                al = h2T_sb[:, ks, r:r + 2, TS:2 * TS]
                                st = (kp == 0)
                                sp = (kp == KH // 2 - 1)
                                for (pp, wh_ap, wl_ap) in (
                                        (pg, wg_pre_hi[:, ks, msl],
                                         wu_t["gl"][:, ks, :]),
                                        (pu, wu_t["h"][:, ks, :],
                                         wu_t["l"][:, ks, :])):
                                    nc.tensor.matmul(pp[:], wh_ap, ah,
                                                     start=st, stop=False,
                                                     perf_mode=DR)
                                    nc.tensor.matmul(pp[:], wl_ap, ah,
                                                     start=False, stop=False,
                                                     perf_mode=DR)
                                    nc.tensor.matmul(pp[:], wh_ap, al,
                                                     start=False, stop=sp,
                                                     perf_mode=DR)
                            gsc = s5.tile([128, 512], F32, tag="gsc",
                                          name=f"gsc{m}_{r}", bufs=3)
                            nc.scalar.activation(gsc[:], pg[:],
                                                 AF.Gelu_apprx_tanh, scale=IWS)
                            tfull = s5.tile([128, 512], BF, tag="tfull",
                                            name=f"tf{m}_{r}", bufs=3)
                            nc.vector.scalar_tensor_tensor(
                                tfull[:], pu[:], IWS, gsc[:],
                                op0=ALU.mult, op1=ALU.mult)
                            csl = slice(r * TS, (r + 2) * TS)
                            nc.vector.tensor_copy(actT_hi[:, m, csl], tfull[:])
                            nc.gpsimd.tensor_sub(actT_lo[:, m, csl], tfull[:],
                                                 actT_hi[:, m, csl])
                    for n in range(5):
                        nsl = slice(n * 512, (n + 1) * 512)
                        wdn_hi = s5.tile([128, MI, 512], F8, tag="wdnh",
                                         name=f"wdnh{n}", bufs=2)
                        wdn_lo = s5.tile([128, MI, 512], F8, tag="wdnl",
                                         name=f"wdnl{n}", bufs=2)
                        nc.sync.dma_start(wdn_hi[:], wd_hi.ap()[:, nsl]
                                          .rearrange("(i p) n -> p i n", p=128))
                        nc.sync.dma_start(wdn_lo[:], wd_lo.ap()[:, nsl]
                                          .rearrange("(i p) n -> p i n", p=128))
                        for tt in range(S // 128):
                            csl = slice(128 * tt, 128 * (tt + 1))
                            pd = psP.tile([128, 512], F32, tag="mm", bufs=6,
                                          name=f"pd{n}_{tt}")
                            for ip in range(MI // 2):
                                isl = slice(2 * ip, 2 * ip + 2)
                                st = (ip == 0)
                                sp = (ip == MI // 2 - 1)
                                nc.tensor.matmul(pd[:], actT_hi[:, isl, csl],
                                                 wdn_hi[:, isl, :], start=st,
                                                 stop=False, perf_mode=DR)
                                nc.tensor.matmul(pd[:], actT_lo[:, isl, csl],
                                                 wdn_hi[:, isl, :], start=False,
                                                 stop=False, perf_mode=DR)
                                nc.tensor.matmul(pd[:], actT_hi[:, isl, csl],
                                                 wdn_lo[:, isl, :], start=False,
                                                 stop=sp, perf_mode=DR)
                            dcp = s5.tile([128, 512], BF, tag="dcp",
                                          name=f"dcp{n}_{tt}", bufs=4)
                            if tt % 2 == 0:
                                nc.vector.tensor_scalar_mul(dcp[:], pd[:], IWS)
                            else:
                                nc.scalar.activation(dcp[:], pd[:], AF.Copy,
                                                     scale=IWS)
                            nc.sync.dma_start(rs2_in[csl, nsl], dcp[:])
                    _coll("ReduceScatter", ALU.add, [rs2_in[:]], [rs2_out[:]])
            wgpre_cm.__exit__(None, None, None)

            mark('S6')
            # ============ S6: post_ff norm + residual ============
            with tc.tile_pool(name="s6", bufs=2) as s6:
                w1_po_b = s6.tile([128, HID], F32, bufs=1)
                _bcast_row(nc, w1_po_b, w1_po, HID)
                for t in range(2):
                    rows = slice(t * 128, (t + 1) * 128)
                    mlp16 = s6.tile([128, HID], BF, tag="mlp", name=f"mlp{t}",
                                    bufs=2)
                    nc.sync.dma_start(mlp16[:], rs2_out[rows, :])
                    x2l = s6.tile([128, HID], F32, tag="x2l", name=f"x2l{t}",
                                  bufs=2)
                    nc.sync.dma_start(x2l[:], x2_spill[rows, :])
                    rinv_o = rmsnorm_rinv(mlp16[:], HID, f"pon{t}")
                    o32 = s6.tile([128, HID], F32, tag="o32", name=f"o32_{t}",
                                  bufs=2)
                    nc.vector.scalar_tensor_tensor(o32[:], mlp16[:], rinv_o[:],
                                                   w1_po_b[:], op0=ALU.mult,
                                                   op1=ALU.mult)
                    nc.vector.tensor_add(o32[:], o32[:], x2l[:])
                    nc.sync.dma_start(out_shard.ap()[rows, :], o32[:])

    nc.compile()
    return nc


_NC_CACHE = None


def _get_nc():
    global _NC_CACHE
    if _NC_CACHE is None:
        _NC_CACHE = build_nc()
    return _NC_CACHE


def _split8(w, scale):
    """fp8 e4m3 hi/lo split of w*scale (hi+lo ~= w*scale to ~fp8^2)."""
    e4m3 = ml_dtypes.float8_e4m3
    ws = np.asarray(w, np.float32) * scale
    hi = ws.astype(e4m3)
    lo = (ws - hi.astype(np.float32)).astype(e4m3)
    return np.ascontiguousarray(hi), np.ascontiguousarray(lo)


def make_in_maps(hidden_states, position_ids, wq, wk, wv, wo, q_ln_w, k_ln_w,
                 in_ln_w, post_attn_ln_w, pre_ff_ln_w, post_ff_ln_w,
                 w_gate, w_up, w_down):
    bf16 = ml_dtypes.bfloat16
    f32 = np.float32
    x = np.asarray(hidden_states, f32).reshape(S, HID)
    pos = np.asarray(position_ids).reshape(S).astype(np.float64)

    inv_freq = 1.0 / (BASE ** (np.arange(0, HD, 2, dtype=np.float64) / HD))
    freqs = pos[:, None] * inv_freq[None, :]
    emb = np.concatenate([freqs, freqs], axis=1)
    cos = np.cos(emb).astype(f32)
    sin = np.sin(emb).astype(f32)
    w1q = 1.0 + np.asarray(q_ln_w, f32)
    w1k = 1.0 + np.asarray(k_ln_w, f32)

    def rope_tabs(w1):
        w1sw = np.concatenate([w1[HALF:], w1[:HALF]])
        sgn = np.concatenate([-np.ones(HALF, f32), np.ones(HALF, f32)])
        return ((cos * w1[None, :]).astype(bf16),
                (sin * (w1sw * sgn)[None, :]).astype(bf16))

    cqw_np, sqw_np = rope_tabs(w1q)
    ckw_np, skw_np = rope_tabs(w1k)

    wqkv = np.concatenate([np.asarray(wq, f32), np.asarray(wk, f32),
                           np.asarray(wv, f32)], axis=1)  # [HID, 4096]
    wqkv_hi, wqkv_lo = _split8(wqkv, WS)
    wo_r = np.asarray(wo, f32).reshape(NH, HD, HID)
    wg_r = np.asarray(w_gate, f32).reshape(HID, NC_, INTER // NC_)
    wu_r = np.asarray(w_up, f32).reshape(HID, NC_, INTER // NC_)
    wd_r = np.asarray(w_down, f32).reshape(NC_, INTER // NC_, HID)

    def _pmajor(w2d):
        # [HID, 1280] -> [MI, 128, KH, 128] (per m-tile, partition-major)
        w4 = np.asarray(w2d).reshape(KH, 128, MI, 128)
        return np.ascontiguousarray(w4.transpose(2, 1, 0, 3))

    common = {
        "wqkv_hi": wqkv_hi, "wqkv_lo": wqkv_lo,
        "w1_in": (1.0 + np.asarray(in_ln_w, f32)).astype(bf16),
        "w1_pa": (1.0 + np.asarray(post_attn_ln_w, f32)).astype(bf16),
        "w1_pf": (1.0 + np.asarray(pre_ff_ln_w, f32)).astype(bf16),
        "w1_po": 1.0 + np.asarray(post_ff_ln_w, f32),
        "cqw": cqw_np, "sqw": sqw_np, "ckw": ckw_np, "skw": skw_np,
    }
    in_maps = []
    for c in range(NC_):
        wo_hi_c, wo_lo_c = _split8(wo_r[c], WS)
        wg_hi_c, wg_lo_c = _split8(wg_r[:, c, :], WS)
        wu_hi_c, wu_lo_c = _split8(wu_r[:, c, :], WS)
        wg_lo_c = _pmajor(wg_lo_c)
        wu_hi_c = _pmajor(wu_hi_c)
        wu_lo_c = _pmajor(wu_lo_c)
        wd_hi_c, wd_lo_c = _split8(wd_r[c], WS)
        in_maps.append({
            "x_shard": np.ascontiguousarray(np.concatenate(
                [x[128 * c:128 * (c + 1)],
                 x[1024 + 128 * c:1024 + 128 * (c + 1)]], axis=0)),
            "wo_hi": wo_hi_c, "wo_lo": wo_lo_c,
            "wg_hi": wg_hi_c, "wg_lo": wg_lo_c,
            "wu_hi": wu_hi_c, "wu_lo": wu_lo_c,
            "wd_hi": wd_hi_c, "wd_lo": wd_lo_c,
            **common,
        })
    return in_maps


def kernel(**inputs):
    in_maps = make_in_maps(**inputs)
    nc = _get_nc()
    res = run_bass_kernel_spmd(nc, in_maps, core_ids=list(range(NC_)))
    out = np.empty((S, HID), np.float32)
    for c in range(NC_):
        sh = res.results[c]["out_shard"]
        out[128 * c:128 * (c + 1)] = sh[0:128]
        out[1024 + 128 * c:1024 + 128 * (c + 1)] = sh[128:256]
    return out.reshape(1, S, HID).astype(np.float32)


# revision 3
# speedup vs baseline: 1.0055x; 1.0055x over previous
"""Gemma3 decoder layer on 8 Trainium2 NeuronCores (Bass/Tile), v2.

Sharding (per core c, kv-group g=c//2):
  - S1: in_ln on own 256 tokens; QKV for own tokens x ALL heads with
    fp8-split3 DoubleRow matmuls (w ~ w_hi+w_lo, a ~ a_hi+a_lo fp8e4;
    compute w_hi@a_hi + w_lo@a_hi + w_hi@a_lo = 1.33x PE throughput at
    better-than-bf16 accuracy); AllToAll -> core c owns Q head c and
    KV head g for the full sequence.
  - S2: per 128-token tile: q/k rmsnorm + rope (tables precomputed on
    host with (1+w) folded), transposes into QT/KT; V direct.
  - S3: sliding-window attention (512) per 512-query block; attn^T is
    split to fp8 and immediately consumed by ROW-parallel wo partial
    matmuls (wo sharded by attn-dim rows = my head); partials for all
    2048 tokens -> ReduceScatter lands my 256 tokens summed.
  - S4: post_attn norm + residual + pre_ff norm; h2^T split to fp8
    hi/lo, AllGather.
  - S5: MLP column/row sharded (1280 inter per core), all matmuls
    fp8-split3 DoubleRow; partial down outputs -> ReduceScatter.
  - S6: post_ff norm + residual.
rmsnorm 1/sqrt via Sqrt activation + DVE reciprocal (one act table per
phase; Ln/Exp thrash removed). Norms/softmax/residual in fp32.
"""
import sys

if "/opt/trn_rl_repo" not in sys.path:
    sys.path.insert(0, "/opt/trn_rl_repo")

import numpy as np
import ml_dtypes

import concourse.bass as bass
import concourse.mybir as mybir
import concourse.tile as tile
from concourse import bacc
from concourse.bass_utils import run_bass_kernel_spmd
from concourse.masks import make_identity

dt = mybir.dt
AF = mybir.ActivationFunctionType
ALU = mybir.AluOpType
BF = dt.bfloat16
F8 = dt.float8e4
F32 = dt.float32
DR = mybir.MatmulPerfMode.DoubleRow

HID, NH, NKV, HD, INTER = 2560, 8, 4, 256, 10240
WIN, EPS, BASE = 512, 1e-6, 10000.0
S = 2048
NC_ = 8
TS = S // NC_              # 256 tokens per core
KH = HID // 128            # 20 hidden-dim chunks
MI = INTER // NC_ // 128   # 10 inter m-tiles per core
HALF = HD // 2
QC = NH * HD + 2 * NKV * HD  # 4096 qkv columns
WS = 64.0                  # fp8 weight scale
IWS = 1.0 / WS


def _bcast_row(nc, sbuf_tile, dram_t, width):
    a = dram_t.ap()
    nc.sync.dma_start(sbuf_tile[:], bass.AP(
        tensor=a.tensor, offset=a.offset, ap=[[0, 128], [1, width]]))


def _swap_ap(src):
    """Read [128, 256] AP with free-dim halves swapped (as [128,2,128])."""
    return bass.AP(tensor=src.tensor, offset=src.offset + HALF,
                   ap=[list(src.ap[0]), [-HALF, 2], [1, HALF]])


def build_nc(sim=False):
    nc = bacc.Bacc("TRN2", target_bir_lowering=False, debug=False,
                   enable_asserts=True, num_devices=1 if sim else NC_)

    def _coll(kind, op, ins, outs):
        if not sim:
            nc.gpsimd.collective_compute(kind, op, replica_groups=rg,
                                         ins=ins, outs=outs)
            return
        i_ap, o_ap = ins[0], outs[0]
        if kind == "AllGather":
            n = i_ap.shape[0]
            for r in range(NC_):
                nc.sync.dma_start(o_ap[r * n:(r + 1) * n], i_ap)
        elif kind == "AllToAll":
            nc.sync.dma_start(o_ap, i_ap)
        elif kind == "ReduceScatter":
            n = o_ap.shape[0]
            nc.sync.dma_start(o_ap, i_ap[0:n])

    x_shard = nc.dram_tensor("x_shard", [TS, HID], F32, kind="ExternalInput")
    wqkv_hi = nc.dram_tensor("wqkv_hi", [HID, QC], F8, kind="ExternalInput")
    wqkv_lo = nc.dram_tensor("wqkv_lo", [HID, QC], F8, kind="ExternalInput")
    wo_w = nc.dram_tensor("wo_w", [HD, HID], BF, kind="ExternalInput")
    wg_hi = nc.dram_tensor("wg_hi", [HID, INTER // NC_], F8, kind="ExternalInput")
    wg_lo = nc.dram_tensor("wg_lo", [MI, 128, KH, 128], F8, kind="ExternalInput")
    wu_hi = nc.dram_tensor("wu_hi", [MI, 128, KH, 128], F8, kind="ExternalInput")
    wu_lo = nc.dram_tensor("wu_lo", [MI, 128, KH, 128], F8, kind="ExternalInput")
    wd_hi = nc.dram_tensor("wd_hi", [INTER // NC_, HID], F8, kind="ExternalInput")
    wd_lo = nc.dram_tensor("wd_lo", [INTER // NC_, HID], F8, kind="ExternalInput")
    w1_in = nc.dram_tensor("w1_in", [HID], BF, kind="ExternalInput")
    w1_pa = nc.dram_tensor("w1_pa", [HID], BF, kind="ExternalInput")
    w1_pf = nc.dram_tensor("w1_pf", [HID], BF, kind="ExternalInput")
    w1_po = nc.dram_tensor("w1_po", [HID], F32, kind="ExternalInput")
    cqw = nc.dram_tensor("cqw", [S, HD], BF, kind="ExternalInput")
    sqw = nc.dram_tensor("sqw", [S, HD], BF, kind="ExternalInput")
    ckw = nc.dram_tensor("ckw", [S, HD], BF, kind="ExternalInput")
    skw = nc.dram_tensor("skw", [S, HD], BF, kind="ExternalInput")
    out_shard = nc.dram_tensor("out_shard", [TS, HID], F32, kind="ExternalOutput")

    rg = [list(range(NC_))]
    stages = {}
    nc._stage_ids = stages

    def mark(name):
        stages[name] = nc.next_id()

    with tile.TileContext(nc) as tc:
        with (
            tc.tile_pool(name="dram", bufs=1, space="DRAM") as dram,
            tc.tile_pool(name="glob", bufs=1) as glob,
            tc.tile_pool(name="nrm", bufs=8) as nrm,
            tc.tile_pool(name="psP", bufs=1, space="PSUM") as psP,
        ):
            # DRAM scratch
            a2a1_in = dram.tile([S, 3 * HD], BF)
            a2a1_out = dram.tile([S, 3 * HD], BF)
            rs1a_in = dram.tile([S // 2, HID], BF)
            rs1a_out = dram.tile([TS // 2, HID], BF)
            rs1b_in = dram.tile([S // 2, HID], BF)
            rs1b_out = dram.tile([TS // 2, HID], BF)
            h2T_in = dram.tile([HID, 2 * TS], F8)
            h2T_full = dram.tile([NC_ * HID, 2 * TS], F8,
                                 addr_space="Local" if sim else "Shared")
            rs2_in = dram.tile([S, HID], BF)
            rs2_out = dram.tile([TS, HID], BF)
            x2_spill = dram.tile([TS, HID], F32)

            ident = glob.tile([128, 128], BF)
            make_identity(nc, ident[:])
            eps_t = glob.tile([128, 1], F32)
            nc.vector.memset(eps_t[:], EPS)

            def rmsnorm_rinv(src_ap, d, name):
                """rinv[p,1] = 1/sqrt(mean(src^2)+EPS): Act Square-accum,
                Act Sqrt(scale=1/d, bias=eps), DVE reciprocal."""
                scr = nrm.tile([128, d], BF, tag=f"nsc{d}", name=f"{name}_sc",
                               bufs=1 if d > 512 else 2)
                ms = nrm.tile([128, 1], F32, tag="nms", name=f"{name}_ms")
                nc.scalar.activation(scr[:], src_ap, AF.Square,
                                     accum_out=ms[:])
                sq = nrm.tile([128, 1], F32, tag="nsq", name=f"{name}_sq")
                nc.scalar.activation(sq[:], ms[:], AF.Sqrt, bias=eps_t[:],
                                     scale=1.0 / d)
                rinv = nrm.tile([128, 1], F32, tag="nrv", name=f"{name}_rv")
                nc.vector.reciprocal(rinv[:], sq[:])
                return rinv

            wgpre_cm = tc.tile_pool(name="wgpre", bufs=1)
            wgpre = wgpre_cm.__enter__()
            wg_pre_hi = wgpre.tile([128, KH, INTER // NC_], F8, name="wg_pre_hi")

            with tc.tile_pool(name="xpool", bufs=1) as xpool:
                x_sb = [xpool.tile([128, HID], F32, name=f"xt{t}") for t in range(2)]

                mark('S1')
                # ============ S1: in_ln + transpose/split + QKV + A2A1 ========
                with tc.tile_pool(name="s1", bufs=2) as s1:
                    w1_in_b = s1.tile([128, HID], BF, bufs=1)
                    _bcast_row(nc, w1_in_b, w1_in, HID)
                    hT_hi = s1.tile([128, KH, TS], F8, bufs=1, name="hT_hi")
                    hT_lo = s1.tile([128, KH, TS], F8, bufs=1, name="hT_lo")
                    qkv16 = [s1.tile([128, QC], BF, bufs=1, name=f"qkv16_{t}")
                             for t in range(2)]
                    h16s = []
                    for t in range(2):
                        nc.sync.dma_start(x_sb[t][:],
                                          x_shard.ap()[t * 128:(t + 1) * 128, :])
                        rinv = rmsnorm_rinv(x_sb[t][:], HID, f"inln{t}")
                        h16 = s1.tile([128, HID], BF, tag="h16", name=f"h16_{t}",
                                      bufs=2)
                        for cch in range(5):
                            sl = slice(cch * 512, (cch + 1) * 512)
                            nc.vector.scalar_tensor_tensor(h16[:, sl], x_sb[t][:, sl],
                                                           rinv[:], w1_in_b[:, sl],
                                                           op0=ALU.mult, op1=ALU.mult)
                        h16s.append(h16)
                    for k in range(KH):
                        ptr = psP.tile([128, TS], BF, tag="tr", bufs=2,
                                       name=f"s1tr{k}")
                        for t in range(2):
                            nc.tensor.transpose(
                                ptr[:, t * 128:(t + 1) * 128],
                                h16s[t][:, k * 128:(k + 1) * 128], ident[:])
                        nc.vector.tensor_copy(hT_hi[:, k, :], ptr[:])
                        nc.vector.tensor_sub(hT_lo[:, k, :], ptr[:], hT_hi[:, k, :])
                    # QKV: full heads for own tokens, fp8 split3
                    for n in range(QC // 512):
                        wn_hi = s1.tile([128, KH, 512], F8, tag="wqn",
                                        name=f"wqh{n}", bufs=3)
                        wn_lo = s1.tile([128, KH, 512], F8, tag="wqnl",
                                        name=f"wql{n}", bufs=3)
                        nc.sync.dma_start(
                            wn_hi[:], wqkv_hi.ap()[:, n * 512:(n + 1) * 512]
                            .rearrange("(k p) n -> p k n", p=128))
                        nc.sync.dma_start(
                            wn_lo[:], wqkv_lo.ap()[:, n * 512:(n + 1) * 512]
                            .rearrange("(k p) n -> p k n", p=128))
                        for t in range(2):
                            pq = psP.tile([128, 512], F32, tag="mm", bufs=6,
                                          name=f"pq{n}_{t}")
                            tsl = slice(t * 128, (t + 1) * 128)
                            for kp in range(KH // 2):
                                ks = slice(2 * kp, 2 * kp + 2)
                                st = (kp == 0)
                                sp = (kp == KH // 2 - 1)
                                nc.tensor.matmul(pq[:], hT_hi[:, ks, tsl],
                                                 wn_hi[:, ks, :], start=st,
                                                 stop=False, perf_mode=DR)
                                nc.tensor.matmul(pq[:], hT_lo[:, ks, tsl],
                                                 wn_hi[:, ks, :], start=False,
                                                 stop=False, perf_mode=DR)
                                nc.tensor.matmul(pq[:], hT_hi[:, ks, tsl],
                                                 wn_lo[:, ks, :], start=False,
                                                 stop=sp, perf_mode=DR)
                            nc.scalar.activation(qkv16[t][:, n * 512:(n + 1) * 512],
                                                 pq[:], AF.Copy, scale=IWS)
                    for d in range(NC_):
                        g = d // 2
                        for t in range(2):
                            rs = slice(TS * d + 128 * t, TS * d + 128 * (t + 1))
                            nc.sync.dma_start(a2a1_in[rs, 0:HD],
                                              qkv16[t][:, HD * d:HD * (d + 1)])
                            nc.sync.dma_start(
                                a2a1_in[rs, HD:2 * HD],
                                qkv16[t][:, NH * HD + HD * g:NH * HD + HD * (g + 1)])
                            nc.sync.dma_start(
                                a2a1_in[rs, 2 * HD:3 * HD],
                                qkv16[t][:, (NH + NKV) * HD + HD * g:
                                          (NH + NKV) * HD + HD * (g + 1)])
                    _coll("AllToAll", ALU.bypass, [a2a1_in[:]], [a2a1_out[:]])
                    nc.sync.dma_start(wg_pre_hi[:], wg_hi.ap()
                                      .rearrange("(k p) n -> p k n", p=128))

                mark('S2')
                # ============ S2/S3: attention + fused row-parallel wo ========
                with tc.tile_pool(name="attp", bufs=1) as attp:
                    ropes = attp.tile([128, 4, S // 128, HD], BF, name="ropes")
                    for ti, tab in enumerate((cqw, sqw, ckw, skw)):
                        nc.sync.dma_start(
                            ropes[:, ti, :, :],
                            tab.ap().rearrange("(t p) d -> p t d", p=128))

                    QTm = attp.tile([128, 2, S], BF, name="QTm")
                    KTm = attp.tile([128, 2, S], BF, name="KTm")
                    V = [attp.tile([128, HD + 1], BF, name=f"V{i}")
                         for i in range(S // 128)]
                    for i in range(S // 128):
                        nc.vector.memset(V[i][:, HD:HD + 1], 1.0)
                    masks = attp.tile([128, 8, 512], BF)
                    for i in range(8):
                        delta = 512 - 128 * i
                        mk = masks[:, i, :]
                        nc.gpsimd.memset(mk, 1.0)
                        nc.gpsimd.affine_select(
                            out=mk, in_=mk, compare_op=ALU.is_ge, fill=0.0,
                            base=delta, pattern=[[1, 512]], channel_multiplier=-1)
                        nc.gpsimd.affine_select(
                            out=mk, in_=mk, compare_op=ALU.is_ge, fill=0.0,
                            base=-delta + (WIN - 1), pattern=[[-1, 512]],
                            channel_multiplier=1)
                    wo_sb = attp.tile([128, 2, HID], BF, name="wo_sb")
                    nc.sync.dma_start(wo_sb[:],
                                      wo_w.ap().rearrange("(a p) n -> p a n", p=128))

                    with tc.tile_pool(name="s2", bufs=2) as s2:
                        for tt in range(S // 128):
                            jj, hh = (tt, 0) if tt < 8 else (tt - 8, 1)
                            base = 256 * jj + 128 * hh
                            rows = slice(base, base + 128)
                            qk_t = s2.tile([128, 2 * HD], BF, tag="qkt",
                                           name=f"qkt{tt}", bufs=3)
                            nc.sync.dma_start(qk_t[:], a2a1_out[rows, 0:2 * HD])
                            nc.sync.dma_start(V[tt][:, 0:HD],
                                              a2a1_out[rows, 2 * HD:3 * HD])
                            for (qo, tbi, QKT, nm) in (
                                    (0, 0, QTm, "q"),
                                    (HD, 2, KTm, "k")):
                                src = qk_t[:, qo:qo + HD]
                                rinv = rmsnorm_rinv(src, HD, f"{nm}n{tt}")
                                ct = ropes[:, tbi, tt, :]
                                st_ = ropes[:, tbi + 1, tt, :]
                                t1 = s2.tile([128, HD], BF, tag="t1",
                                             name=f"t1{nm}{tt}", bufs=3)
                                t2 = s2.tile([128, HD], BF, tag="t2",
                                             name=f"t2{nm}{tt}", bufs=3)
                                nc.vector.scalar_tensor_tensor(
                                    t1[:], src, rinv[:], ct,
                                    op0=ALU.mult, op1=ALU.mult)
                                nc.gpsimd.tensor_mul(
                                    t2[:].rearrange("p (a b) -> p a b", a=2),
                                    _swap_ap(src),
                                    st_.rearrange("p (a b) -> p a b", a=2))
                                qr = s2.tile([128, HD], BF, tag="qr",
                                             name=f"qr{nm}{tt}", bufs=3)
                                nc.vector.scalar_tensor_tensor(
                                    qr[:], t2[:], rinv[:], t1[:],
                                    op0=ALU.mult, op1=ALU.add)
                                ptr = psP.tile([128, HD], BF, tag="tr",
                                               bufs=2, name=f"s2t{nm}{tt}")
                                for h in range(2):
                                    nc.tensor.transpose(
                                        ptr[:, h * 128:(h + 1) * 128],
                                        qr[:, h * 128:(h + 1) * 128], ident[:])
                                nc.vector.tensor_copy(
                                    QKT[:, :, 128 * tt:128 * (tt + 1)],
                                    ptr[:].rearrange("p (a b) -> p a b", a=2))

                    mark('S3')
                    with tc.tile_pool(name="s3", bufs=2) as s3:
                        w1_pa_b = s3.tile([128, HID], BF, bufs=1, name="w1pab")
                        w1_pf_b = s3.tile([128, HID], BF, bufs=1, name="w1pfb")
                        _bcast_row(nc, w1_pa_b, w1_pa, HID)
                        _bcast_row(nc, w1_pf_b, w1_pf, HID)
                        h2Ts = s3.tile([128, KH, 2 * TS], F8, bufs=1, name="h2Ts")

                        def s4_half(t, rs_out_t):
                            ao = s3.tile([128, HID], BF, tag="ao", name=f"ao{t}",
                                         bufs=2)
                            nc.sync.dma_start(ao[:], rs_out_t[:])
                            rinv_a = rmsnorm_rinv(ao[:], HID, f"pan{t}")
                            x2 = s3.tile([128, HID], F32, tag="x2",
                                         name=f"x2_{t}", bufs=1)
                            for cch in range(5):
                                sl = slice(cch * 512, (cch + 1) * 512)
                                nc.vector.scalar_tensor_tensor(
                                    x2[:, sl], ao[:, sl], rinv_a[:],
                                    w1_pa_b[:, sl], op0=ALU.mult, op1=ALU.mult)
                                nc.gpsimd.tensor_add(x2[:, sl], x2[:, sl],
                                                      x_sb[t][:, sl])
                            nc.sync.dma_start(
                                x2_spill[t * 128:(t + 1) * 128, :], x2[:])
                            rinv_f = rmsnorm_rinv(x2[:], HID, f"pff{t}")
                            h2 = s3.tile([128, HID], BF, tag="h2", name=f"h2_{t}",
                                         bufs=2)
                            for cch in range(5):
                                sl = slice(cch * 512, (cch + 1) * 512)
                                nc.vector.scalar_tensor_tensor(
                                    h2[:, sl], x2[:, sl], rinv_f[:],
                                    w1_pf_b[:, sl], op0=ALU.mult, op1=ALU.mult)
                            for k in range(KH):
                                ptr = psP.tile([128, 128], BF, tag="tr", bufs=2,
                                               name=f"s4tr{k}_{t}")
                                nc.tensor.transpose(
                                    ptr[:], h2[:, k * 128:(k + 1) * 128],
                                    ident[:])
                                tsl = slice(t * 128, (t + 1) * 128)
                                lsl = slice(TS + t * 128, TS + (t + 1) * 128)
                                nc.vector.tensor_copy(h2Ts[:, k, tsl], ptr[:])
                                nc.vector.tensor_sub(h2Ts[:, k, lsl], ptr[:],
                                                     h2Ts[:, k, tsl])

                        for qb in range(4):
                            q0 = 512 * qb
                            probs = {}
                            for i in range(8):
                                kc = q0 - 512 + 128 * i
                                if kc < 0:
                                    continue
                                psc = psP.tile([128, 512], F32, tag="mm", bufs=6,
                                               name=f"psc{qb}_{i}")
                                for h in range(2):
                                    nc.tensor.matmul(psc[:], KTm[:, h, kc:kc + 128],
                                                     QTm[:, h, q0:q0 + 512],
                                                     start=(h == 0), stop=(h == 1))
                                pr = s3.tile([128, 512], BF, tag="pr",
                                             name=f"pr{qb}_{i}", bufs=10)
                                nc.scalar.activation(pr[:], psc[:], AF.Exp,
                                                     scale=1.0 / 16.0)
                                nc.gpsimd.tensor_mul(pr[:], pr[:], masks[:, i, :])
                                probs[kc] = pr
                            for qs in range(4):
                                qa = q0 + 128 * qs
                                kcs = [kc for kc in range(qa - 512, qa + 128, 128)
                                       if kc >= 0]
                                po = psP.tile([128, HD + 1], F32, tag="mm", bufs=6,
                                              name=f"po{qb}_{qs}")
                                col = qa - q0
                                for j, kc in enumerate(kcs):
                                    nc.tensor.matmul(po[:],
                                                     probs[kc][:, col:col + 128],
                                                     V[kc // 128][:], start=(j == 0),
                                                     stop=(j == len(kcs) - 1))
                                rec = s3.tile([128, 1], F32, tag="rec",
                                              name=f"rec{qb}_{qs}")
                                nc.vector.reciprocal(rec[:], po[:, HD:HD + 1])
                                an = s3.tile([128, HD], BF, tag="an",
                                             name=f"an{qb}_{qs}")
                                nc.vector.tensor_scalar_mul(an[:], po[:, 0:HD],
                                                            rec[:])
                                ptr = psP.tile([128, HD], BF, tag="tr", bufs=2,
                                               name=f"s3tr{qb}{qs}")
                                for h in range(2):
                                    nc.tensor.transpose(
                                        ptr[:, h * 128:(h + 1) * 128],
                                        an[:, h * 128:(h + 1) * 128], ident[:])
                                aT = s3.tile([128, 2, 128], BF, tag="aTh",
                                             name=f"aTh{qb}_{qs}", bufs=3)
                                p3 = ptr[:].rearrange("p (a b) -> p a b", a=2)
                                nc.vector.tensor_copy(aT[:], p3)
                                # row-parallel wo partials for this token tile
                                xo = s3.tile([128, HID], BF, tag="xo",
                                             name=f"xo{qb}_{qs}", bufs=3)
                                for n in range(5):
                                    pw = psP.tile([128, 512], F32, tag="mm",
                                                  bufs=6, name=f"pw{qb}{qs}_{n}")
                                    nsl = slice(n * 512, (n + 1) * 512)
                                    for h in range(2):
                                        nc.tensor.matmul(pw[:], aT[:, h, :],
                                                         wo_sb[:, h, nsl],
                                                         start=(h == 0),
                                                         stop=(h == 1))
                                    if n % 2 == 0:
                                        nc.scalar.activation(xo[:, nsl], pw[:],
                                                             AF.Copy)
                                    else:
                                        nc.vector.tensor_copy(xo[:, nsl], pw[:])
                                qq = qa // 128
                                if qq < 8:
                                    nc.sync.dma_start(
                                        rs1a_in[128 * qq:128 * (qq + 1), :], xo[:])
                                else:
                                    nc.sync.dma_start(
                                        rs1b_in[128 * (qq - 8):128 * (qq - 7), :],
                                        xo[:])
                            if qb == 1:
                                _coll("ReduceScatter", ALU.add, [rs1a_in[:]],
                                      [rs1a_out[:]])
                                s4_half(0, rs1a_out)
                            elif qb == 3:
                                _coll("ReduceScatter", ALU.add, [rs1b_in[:]],
                                      [rs1b_out[:]])
                                s4_half(1, rs1b_out)
                        nc.sync.dma_start(
                            h2T_in[:].rearrange("(k p) t -> p k t", p=128),
                            h2Ts[:])
                        if not sim:
                            _coll("AllGather", ALU.bypass, [h2T_in[:]],
                                  [h2T_full[:]])

            mark('S5')
            # ================= S5: MLP (fp8 split3) =================
            with tc.tile_pool(name="s5w", bufs=1) as s5w:
                h2T_sb = s5w.tile([128, KH, NC_, 2 * TS], F8, name="h2T_sb")
                actT_hi = s5w.tile([128, MI, S], F8, name="actT_hi")
                actT_lo = s5w.tile([128, MI, S], F8, name="actT_lo")
                with tc.tile_pool(name="s5", bufs=2) as s5:
                    def load_wu(m):
                        d = {}
                        for (sfx, t_) in (("h", wu_hi), ("l", wu_lo),
                                          ("gl", wg_lo)):
                            w_ = s5.tile([128, KH, 128], F8, tag=f"wu{sfx}",
                                         name=f"wu{sfx}{m}", bufs=2)
                            nc.sync.dma_start(w_[:], t_.ap()[m])
                            d[sfx] = w_
                        return d

                    wu_tiles = {0: load_wu(0)}
                    for r in range(NC_):
                        blk = h2T_full[r * HID:(r + 1) * HID]
                        if sim:
                            nc.sync.dma_start(blk, h2T_in[:])
                        nc.sync.dma_start(
                            h2T_sb[:, :, r, :],
                            blk.rearrange("(k p) t -> p k t", p=128))
                        if r == 1:
                            wu_tiles[1] = load_wu(1)
                    for m in range(MI):
                        msl = slice(m * 128, (m + 1) * 128)
                        wu_t = wu_tiles.pop(m)
                        if m + 2 < MI:
                            wu_tiles[m + 2] = load_wu(m + 2)
                        for r in range(0, NC_, 2):
                            pg = psP.tile([128, 512], F32, tag="mm", bufs=6,
                                          name=f"pg{m}_{r}")
                            pu = psP.tile([128, 512], F32, tag="mm", bufs=6,
                                          name=f"pu{m}_{r}")
                            for kp in range(KH // 2):
                                ks = slice(2 * kp, 2 * kp + 2)
                                ah = h2T_sb[:, ks, r:r + 2, 0:TS]
                                al = h2T_sb[:, ks, r:r + 2, TS:2 * TS]
                                st = (kp == 0)
                                sp = (kp == KH // 2 - 1)
                                for (pp, wh_ap, wl_ap) in (
                                        (pg, wg_pre_hi[:, ks, msl],
                                         wu_t["gl"][:, ks, :]),
                                        (pu, wu_t["h"][:, ks, :],
                                         wu_t["l"][:, ks, :])):
                                    nc.tensor.matmul(pp[:], wh_ap, ah,
                                                     start=st, stop=False,
                                                     perf_mode=DR)
                                    nc.tensor.matmul(pp[:], wl_ap, ah,
                                                     start=False, stop=False,
                                                     perf_mode=DR)
                                    nc.tensor.matmul(pp[:], wh_ap, al,
                                                     start=False, stop=sp,
                                                     perf_mode=DR)
                            gsc = s5.tile([128, 512], F32, tag="gsc",
                                          name=f"gsc{m}_{r}", bufs=3)
                            nc.scalar.activation(gsc[:], pg[:],
                                                 AF.Gelu_apprx_tanh, scale=IWS)
                            tfull = s5.tile([128, 512], BF, tag="tfull",
                                            name=f"tf{m}_{r}", bufs=3)
                            nc.vector.scalar_tensor_tensor(
                                tfull[:], pu[:], IWS, gsc[:],
                                op0=ALU.mult, op1=ALU.mult)
                            csl = slice(r * TS, (r + 2) * TS)
                            nc.vector.tensor_copy(actT_hi[:, m, csl], tfull[:])
                            nc.gpsimd.tensor_sub(actT_lo[:, m, csl], tfull[:],
                                                 actT_hi[:, m, csl])
                    for n in range(5):
                        nsl = slice(n * 512, (n + 1) * 512)
                        wdn_hi = s5.tile([128, MI, 512], F8, tag="wdnh",
                                         name=f"wdnh{n}", bufs=2)
                        wdn_lo = s5.tile([128, MI, 512], F8, tag="wdnl",
                                         name=f"wdnl{n}", bufs=2)
                        nc.sync.dma_start(wdn_hi[:], wd_hi.ap()[:, nsl]
                                          .rearrange("(i p) n -> p i n", p=128))
                        nc.sync.dma_start(wdn_lo[:], wd_lo.ap()[:, nsl]
                                          .rearrange("(i p) n -> p i n", p=128))
                        for tt in range(S // 128):
                            csl = slice(128 * tt, 128 * (tt + 1))
                            pd = psP.tile([128, 512], F32, tag="mm", bufs=6,
                                          name=f"pd{n}_{tt}")
                            for ip in range(MI // 2):
                                isl = slice(2 * ip, 2 * ip + 2)
                                st = (ip == 0)
                                sp = (ip == MI // 2 - 1)
                                nc.tensor.matmul(pd[:], actT_hi[:, isl, csl],
                                                 wdn_hi[:, isl, :], start=st,
                                                 stop=False, perf_mode=DR)
                                nc.tensor.matmul(pd[:], actT_lo[:, isl, csl],
                                                 wdn_hi[:, isl, :], start=False,
                                                 stop=False, perf_mode=DR)
                                nc.tensor.matmul(pd[:], actT_hi[:, isl, csl],
                                                 wdn_lo[:, isl, :], start=False,
                                                 stop=sp, perf_mode=DR)
                            dcp = s5.tile([128, 512], BF, tag="dcp",
                                          name=f"dcp{n}_{tt}", bufs=4)
                            if tt % 2 == 0:
                                nc.vector.tensor_scalar_mul(dcp[:], pd[:], IWS)
                            else:
                                nc.scalar.activation(dcp[:], pd[:], AF.Copy,
                                                     scale=IWS)
                            nc.sync.dma_start(rs2_in[csl, nsl], dcp[:])
                    _coll("ReduceScatter", ALU.add, [rs2_in[:]], [rs2_out[:]])
            wgpre_cm.__exit__(None, None, None)

            mark('S6')
            # ============ S6: post_ff norm + residual ============
            with tc.tile_pool(name="s6", bufs=2) as s6:
                w1_po_b = s6.tile([128, HID], F32, bufs=1)
                _bcast_row(nc, w1_po_b, w1_po, HID)
                for t in range(2):
                    rows = slice(t * 128, (t + 1) * 128)
                    mlp16 = s6.tile([128, HID], BF, tag="mlp", name=f"mlp{t}",
                                    bufs=2)
                    nc.sync.dma_start(mlp16[:], rs2_out[rows, :])
                    x2l = s6.tile([128, HID], F32, tag="x2l", name=f"x2l{t}",
                                  bufs=2)
                    nc.sync.dma_start(x2l[:], x2_spill[rows, :])
                    rinv_o = rmsnorm_rinv(mlp16[:], HID, f"pon{t}")
                    o32 = s6.tile([128, HID], F32, tag="o32", name=f"o32_{t}",
                                  bufs=2)
                    nc.vector.scalar_tensor_tensor(o32[:], mlp16[:], rinv_o[:],
                                                   w1_po_b[:], op0=ALU.mult,
                                                   op1=ALU.mult)
                    nc.vector.tensor_add(o32[:], o32[:], x2l[:])
                    nc.sync.dma_start(out_shard.ap()[rows, :], o32[:])

    nc.compile()
    return nc


_NC_CACHE = None


def _get_nc():
    global _NC_CACHE
    if _NC_CACHE is None:
        _NC_CACHE = build_nc()
    return _NC_CACHE


def _split8(w, scale):
    """fp8 e4m3 hi/lo split of w*scale (hi+lo ~= w*scale to ~fp8^2)."""
    e4m3 = ml_dtypes.float8_e4m3
    ws = np.asarray(w, np.float32) * scale
    hi = ws.astype(e4m3)
    lo = (ws - hi.astype(np.float32)).astype(e4m3)
    return np.ascontiguousarray(hi), np.ascontiguousarray(lo)


def make_in_maps(hidden_states, position_ids, wq, wk, wv, wo, q_ln_w, k_ln_w,
                 in_ln_w, post_attn_ln_w, pre_ff_ln_w, post_ff_ln_w,
                 w_gate, w_up, w_down):
    bf16 = ml_dtypes.bfloat16
    f32 = np.float32
    x = np.asarray(hidden_states, f32).reshape(S, HID)
    pos = np.asarray(position_ids).reshape(S).astype(np.float64)

    inv_freq = 1.0 / (BASE ** (np.arange(0, HD, 2, dtype=np.float64) / HD))
    freqs = pos[:, None] * inv_freq[None, :]
    emb = np.concatenate([freqs, freqs], axis=1)
    cos = np.cos(emb).astype(f32)
    sin = np.sin(emb).astype(f32)
    w1q = 1.0 + np.asarray(q_ln_w, f32)
    w1k = 1.0 + np.asarray(k_ln_w, f32)

    def rope_tabs(w1):
        w1sw = np.concatenate([w1[HALF:], w1[:HALF]])
        sgn = np.concatenate([-np.ones(HALF, f32), np.ones(HALF, f32)])
        return ((cos * w1[None, :]).astype(bf16),
                (sin * (w1sw * sgn)[None, :]).astype(bf16))

    cqw_np, sqw_np = rope_tabs(w1q)
    ckw_np, skw_np = rope_tabs(w1k)

    wqkv = np.concatenate([np.asarray(wq, f32), np.asarray(wk, f32),
                           np.asarray(wv, f32)], axis=1)  # [HID, 4096]
    wqkv_hi, wqkv_lo = _split8(wqkv, WS)
    bf16_t = ml_dtypes.bfloat16
    wo_r = np.asarray(wo, f32).reshape(NH, HD, HID)
    wg_r = np.asarray(w_gate, f32).reshape(HID, NC_, INTER // NC_)
    wu_r = np.asarray(w_up, f32).reshape(HID, NC_, INTER // NC_)
    wd_r = np.asarray(w_down, f32).reshape(NC_, INTER // NC_, HID)

    def _pmajor(w2d):
        # [HID, 1280] -> [MI, 128, KH, 128] (per m-tile, partition-major)
        w4 = np.asarray(w2d).reshape(KH, 128, MI, 128)
        return np.ascontiguousarray(w4.transpose(2, 1, 0, 3))

    common = {
        "wqkv_hi": wqkv_hi, "wqkv_lo": wqkv_lo,
        "w1_in": (1.0 + np.asarray(in_ln_w, f32)).astype(bf16),
        "w1_pa": (1.0 + np.asarray(post_attn_ln_w, f32)).astype(bf16),
        "w1_pf": (1.0 + np.asarray(pre_ff_ln_w, f32)).astype(bf16),
        "w1_po": 1.0 + np.asarray(post_ff_ln_w, f32),
        "cqw": cqw_np, "sqw": sqw_np, "ckw": ckw_np, "skw": skw_np,
    }
    in_maps = []
    for c in range(NC_):
        wg_hi_c, wg_lo_c = _split8(wg_r[:, c, :], WS)
        wu_hi_c, wu_lo_c = _split8(wu_r[:, c, :], WS)
        wg_lo_c = _pmajor(wg_lo_c)
        wu_hi_c = _pmajor(wu_hi_c)
        wu_lo_c = _pmajor(wu_lo_c)
        wd_hi_c, wd_lo_c = _split8(wd_r[c], WS)
        in_maps.append({
            "x_shard": np.ascontiguousarray(np.concatenate(
                [x[128 * c:128 * (c + 1)],
                 x[1024 + 128 * c:1024 + 128 * (c + 1)]], axis=0)),
            "wo_w": np.ascontiguousarray(wo_r[c]).astype(bf16_t),
            "wg_hi": wg_hi_c, "wg_lo": wg_lo_c,
            "wu_hi": wu_hi_c, "wu_lo": wu_lo_c,
            "wd_hi": wd_hi_c, "wd_lo": wd_lo_c,
            **common,
        })
    return in_maps


def kernel(**inputs):
    in_maps = make_in_maps(**inputs)
    nc = _get_nc()
    res = run_bass_kernel_spmd(nc, in_maps, core_ids=list(range(NC_)))
    out = np.empty((S, HID), np.float32)
    for c in range(NC_):
        sh = res.results[c]["out_shard"]
        out[128 * c:128 * (c + 1)] = sh[0:128]
        out[1024 + 128 * c:1024 + 128 * (c + 1)] = sh[128:256]
    return out.reshape(1, S, HID).astype(np.float32)


# revision 4
# speedup vs baseline: 1.0082x; 1.0028x over previous
"""Gemma3 decoder layer on 8 Trainium2 NeuronCores (Bass/Tile), v2.

Sharding (per core c, kv-group g=c//2):
  - S1: in_ln on own 256 tokens; QKV for own tokens x ALL heads with
    fp8-split3 DoubleRow matmuls (w ~ w_hi+w_lo, a ~ a_hi+a_lo fp8e4;
    compute w_hi@a_hi + w_lo@a_hi + w_hi@a_lo = 1.33x PE throughput at
    better-than-bf16 accuracy); AllToAll -> core c owns Q head c and
    KV head g for the full sequence.
  - S2: per 128-token tile: q/k rmsnorm + rope (tables precomputed on
    host with (1+w) folded), transposes into QT/KT; V direct.
  - S3: sliding-window attention (512) per 512-query block; attn^T is
    split to fp8 and immediately consumed by ROW-parallel wo partial
    matmuls (wo sharded by attn-dim rows = my head); partials for all
    2048 tokens -> ReduceScatter lands my 256 tokens summed.
  - S4: post_attn norm + residual + pre_ff norm; h2^T split to fp8
    hi/lo, AllGather.
  - S5: MLP column/row sharded (1280 inter per core), all matmuls
    fp8-split3 DoubleRow; partial down outputs -> ReduceScatter.
  - S6: post_ff norm + residual.
rmsnorm 1/sqrt via Sqrt activation + DVE reciprocal (one act table per
phase; Ln/Exp thrash removed). Norms/softmax/residual in fp32.
"""
import sys

if "/opt/trn_rl_repo" not in sys.path:
    sys.path.insert(0, "/opt/trn_rl_repo")

import numpy as np
import ml_dtypes

import concourse.bass as bass
import concourse.mybir as mybir
import concourse.tile as tile
from concourse import bacc
from concourse.bass_utils import run_bass_kernel_spmd
from concourse.masks import make_identity

dt = mybir.dt
AF = mybir.ActivationFunctionType
ALU = mybir.AluOpType
BF = dt.bfloat16
F8 = dt.float8e4
F32 = dt.float32
DR = mybir.MatmulPerfMode.DoubleRow

HID, NH, NKV, HD, INTER = 2560, 8, 4, 256, 10240
WIN, EPS, BASE = 512, 1e-6, 10000.0
S = 2048
NC_ = 8
TS = S // NC_              # 256 tokens per core
KH = HID // 128            # 20 hidden-dim chunks
MI = INTER // NC_ // 128   # 10 inter m-tiles per core
HALF = HD // 2
QC = NH * HD + 2 * NKV * HD  # 4096 qkv columns
WS = 64.0                  # fp8 weight scale
IWS = 1.0 / WS


def _bcast_row(nc, sbuf_tile, dram_t, width):
    a = dram_t.ap()
    nc.sync.dma_start(sbuf_tile[:], bass.AP(
        tensor=a.tensor, offset=a.offset, ap=[[0, 128], [1, width]]))


def _swap_ap(src):
    """Read [128, 256] AP with free-dim halves swapped (as [128,2,128])."""
    return bass.AP(tensor=src.tensor, offset=src.offset + HALF,
                   ap=[list(src.ap[0]), [-HALF, 2], [1, HALF]])


def build_nc(sim=False):
    nc = bacc.Bacc("TRN2", target_bir_lowering=False, debug=False,
                   enable_asserts=True, num_devices=1 if sim else NC_)

    def _coll(kind, op, ins, outs):
        if not sim:
            nc.gpsimd.collective_compute(kind, op, replica_groups=rg,
                                         ins=ins, outs=outs)
            return
        i_ap, o_ap = ins[0], outs[0]
        if kind == "AllGather":
            n = i_ap.shape[0]
            for r in range(NC_):
                nc.sync.dma_start(o_ap[r * n:(r + 1) * n], i_ap)
        elif kind == "AllToAll":
            nc.sync.dma_start(o_ap, i_ap)
        elif kind == "ReduceScatter":
            n = o_ap.shape[0]
            nc.sync.dma_start(o_ap, i_ap[0:n])

    x_shard = nc.dram_tensor("x_shard", [TS, HID], F32, kind="ExternalInput")
    wqkv_hi = nc.dram_tensor("wqkv_hi", [HID, QC], F8, kind="ExternalInput")
    wqkv_lo = nc.dram_tensor("wqkv_lo", [HID, QC], F8, kind="ExternalInput")
    wo_w = nc.dram_tensor("wo_w", [HD, HID], BF, kind="ExternalInput")
    wg_hi = nc.dram_tensor("wg_hi", [HID, INTER // NC_], F8, kind="ExternalInput")
    wg_lo = nc.dram_tensor("wg_lo", [MI, 128, KH, 128], F8, kind="ExternalInput")
    wu_hi = nc.dram_tensor("wu_hi", [MI, 128, KH, 128], F8, kind="ExternalInput")
    wu_lo = nc.dram_tensor("wu_lo", [MI, 128, KH, 128], F8, kind="ExternalInput")
    wd_hi = nc.dram_tensor("wd_hi", [INTER // NC_, HID], F8, kind="ExternalInput")
    wd_lo = nc.dram_tensor("wd_lo", [INTER // NC_, HID], F8, kind="ExternalInput")
    w1_in = nc.dram_tensor("w1_in", [HID], BF, kind="ExternalInput")
    w1_pa = nc.dram_tensor("w1_pa", [HID], BF, kind="ExternalInput")
    w1_pf = nc.dram_tensor("w1_pf", [HID], BF, kind="ExternalInput")
    w1_po = nc.dram_tensor("w1_po", [HID], F32, kind="ExternalInput")
    cqw = nc.dram_tensor("cqw", [S, HD], BF, kind="ExternalInput")
    sqw = nc.dram_tensor("sqw", [S, HD], BF, kind="ExternalInput")
    ckw = nc.dram_tensor("ckw", [S, HD], BF, kind="ExternalInput")
    skw = nc.dram_tensor("skw", [S, HD], BF, kind="ExternalInput")
    out_shard = nc.dram_tensor("out_shard", [TS, HID], F32, kind="ExternalOutput")

    rg = [list(range(NC_))]
    stages = {}
    nc._stage_ids = stages

    def mark(name):
        stages[name] = nc.next_id()

    with tile.TileContext(nc) as tc:
        with (
            tc.tile_pool(name="dram", bufs=1, space="DRAM") as dram,
            tc.tile_pool(name="glob", bufs=1) as glob,
            tc.tile_pool(name="nrm", bufs=8) as nrm,
            tc.tile_pool(name="psP", bufs=1, space="PSUM") as psP,
        ):
            # DRAM scratch
            a2aq_in = dram.tile([S, HD], BF)
            a2aq_out = dram.tile([S, HD], BF)
            a2akv_in = dram.tile([S, 2 * HD], BF)
            a2akv_out = dram.tile([S, 2 * HD], BF)
            rs1a_in = dram.tile([S // 2, HID], BF)
            rs1a_out = dram.tile([TS // 2, HID], BF)
            rs1b_in = dram.tile([S // 2, HID], BF)
            rs1b_out = dram.tile([TS // 2, HID], BF)
            h2T_in = dram.tile([HID, 2 * TS], F8)
            h2T_full = dram.tile([NC_ * HID, 2 * TS], F8,
                                 addr_space="Local" if sim else "Shared")
            rs2_in = dram.tile([S, HID], BF)
            rs2_out = dram.tile([TS, HID], BF)
            x2_spill = dram.tile([TS, HID], F32)

            ident = glob.tile([128, 128], BF)
            make_identity(nc, ident[:])
            eps_t = glob.tile([128, 1], F32)
            nc.vector.memset(eps_t[:], EPS)

            def rmsnorm_rinv(src_ap, d, name):
                """rinv[p,1] = 1/sqrt(mean(src^2)+EPS): Act Square-accum,
                Act Sqrt(scale=1/d, bias=eps), DVE reciprocal."""
                scr = nrm.tile([128, d], BF, tag=f"nsc{d}", name=f"{name}_sc",
                               bufs=1 if d > 512 else 2)
                ms = nrm.tile([128, 1], F32, tag="nms", name=f"{name}_ms")
                nc.scalar.activation(scr[:], src_ap, AF.Square,
                                     accum_out=ms[:])
                sq = nrm.tile([128, 1], F32, tag="nsq", name=f"{name}_sq")
                nc.scalar.activation(sq[:], ms[:], AF.Sqrt, bias=eps_t[:],
                                     scale=1.0 / d)
                rinv = nrm.tile([128, 1], F32, tag="nrv", name=f"{name}_rv")
                nc.vector.reciprocal(rinv[:], sq[:])
                return rinv

            wgpre_cm = tc.tile_pool(name="wgpre", bufs=1)
            wgpre = wgpre_cm.__enter__()
            wg_pre_hi = wgpre.tile([128, KH, INTER // NC_], F8, name="wg_pre_hi")

            with tc.tile_pool(name="xpool", bufs=1) as xpool:
                x_sb = [xpool.tile([128, HID], F32, name=f"xt{t}") for t in range(2)]

                mark('S1')
                # ============ S1: in_ln + transpose/split + QKV + A2A1 ========
                with tc.tile_pool(name="s1", bufs=2) as s1:
                    w1_in_b = s1.tile([128, HID], BF, bufs=1)
                    _bcast_row(nc, w1_in_b, w1_in, HID)
                    hT_hi = s1.tile([128, KH, TS], F8, bufs=1, name="hT_hi")
                    hT_lo = s1.tile([128, KH, TS], F8, bufs=1, name="hT_lo")
                    qkv16 = [s1.tile([128, QC], BF, bufs=1, name=f"qkv16_{t}")
                             for t in range(2)]
                    h16s = []
                    for t in range(2):
                        nc.sync.dma_start(x_sb[t][:],
                                          x_shard.ap()[t * 128:(t + 1) * 128, :])
                        rinv = rmsnorm_rinv(x_sb[t][:], HID, f"inln{t}")
                        h16 = s1.tile([128, HID], BF, tag="h16", name=f"h16_{t}",
                                      bufs=2)
                        for cch in range(5):
                            sl = slice(cch * 512, (cch + 1) * 512)
                            nc.vector.scalar_tensor_tensor(h16[:, sl], x_sb[t][:, sl],
                                                           rinv[:], w1_in_b[:, sl],
                                                           op0=ALU.mult, op1=ALU.mult)
                        h16s.append(h16)
                    for k in range(KH):
                        ptr = psP.tile([128, TS], BF, tag="tr", bufs=2,
                                       name=f"s1tr{k}")
                        for t in range(2):
                            nc.tensor.transpose(
                                ptr[:, t * 128:(t + 1) * 128],
                                h16s[t][:, k * 128:(k + 1) * 128], ident[:])
                        nc.vector.tensor_copy(hT_hi[:, k, :], ptr[:])
                        nc.vector.tensor_sub(hT_lo[:, k, :], ptr[:], hT_hi[:, k, :])
                    # QKV: full heads for own tokens, fp8 split3.
                    # K/V column chunks first so their AllToAll overlaps the
                    # Q-chunk matmuls.
                    for n in (4, 5, 6, 7, 0, 1, 2, 3):
                        wn_hi = s1.tile([128, KH, 512], F8, tag="wqn",
                                        name=f"wqh{n}", bufs=3)
                        wn_lo = s1.tile([128, KH, 512], F8, tag="wqnl",
                                        name=f"wql{n}", bufs=3)
                        nc.sync.dma_start(
                            wn_hi[:], wqkv_hi.ap()[:, n * 512:(n + 1) * 512]
                            .rearrange("(k p) n -> p k n", p=128))
                        nc.sync.dma_start(
                            wn_lo[:], wqkv_lo.ap()[:, n * 512:(n + 1) * 512]
                            .rearrange("(k p) n -> p k n", p=128))
                        for t in range(2):
                            pq = psP.tile([128, 512], F32, tag="mm", bufs=6,
                                          name=f"pq{n}_{t}")
                            tsl = slice(t * 128, (t + 1) * 128)
                            for kp in range(KH // 2):
                                ks = slice(2 * kp, 2 * kp + 2)
                                st = (kp == 0)
                                sp = (kp == KH // 2 - 1)
                                nc.tensor.matmul(pq[:], hT_hi[:, ks, tsl],
                                                 wn_hi[:, ks, :], start=st,
                                                 stop=False, perf_mode=DR)
                                nc.tensor.matmul(pq[:], hT_lo[:, ks, tsl],
                                                 wn_hi[:, ks, :], start=False,
                                                 stop=False, perf_mode=DR)
                                nc.tensor.matmul(pq[:], hT_hi[:, ks, tsl],
                                                 wn_lo[:, ks, :], start=False,
                                                 stop=sp, perf_mode=DR)
                            nc.scalar.activation(qkv16[t][:, n * 512:(n + 1) * 512],
                                                 pq[:], AF.Copy, scale=IWS)
                        if n == 7:
                            for d in range(NC_):
                                g = d // 2
                                for t in range(2):
                                    rs = slice(TS * d + 128 * t,
                                               TS * d + 128 * (t + 1))
                                    nc.sync.dma_start(
                                        a2akv_in[rs, 0:HD],
                                        qkv16[t][:, NH * HD + HD * g:
                                                  NH * HD + HD * (g + 1)])
                                    nc.sync.dma_start(
                                        a2akv_in[rs, HD:2 * HD],
                                        qkv16[t][:, (NH + NKV) * HD + HD * g:
                                                  (NH + NKV) * HD + HD * (g + 1)])
                            _coll("AllToAll", ALU.bypass, [a2akv_in[:]],
                                  [a2akv_out[:]])
                    for d in range(NC_):
                        for t in range(2):
                            rs = slice(TS * d + 128 * t, TS * d + 128 * (t + 1))
                            nc.sync.dma_start(a2aq_in[rs, :],
                                              qkv16[t][:, HD * d:HD * (d + 1)])
                    _coll("AllToAll", ALU.bypass, [a2aq_in[:]], [a2aq_out[:]])
                    nc.sync.dma_start(wg_pre_hi[:], wg_hi.ap()
                                      .rearrange("(k p) n -> p k n", p=128))

                mark('S2')
                # ============ S2/S3: attention + fused row-parallel wo ========
                with tc.tile_pool(name="attp", bufs=1) as attp:
                    ropes = attp.tile([128, 4, S // 128, HD], BF, name="ropes")
                    for ti, tab in enumerate((cqw, sqw, ckw, skw)):
                        nc.sync.dma_start(
                            ropes[:, ti, :, :],
                            tab.ap().rearrange("(t p) d -> p t d", p=128))

                    QTm = attp.tile([128, 2, S], BF, name="QTm")
                    KTm = attp.tile([128, 2, S], BF, name="KTm")
                    V = [attp.tile([128, HD + 1], BF, name=f"V{i}")
                         for i in range(S // 128)]
                    for i in range(S // 128):
                        nc.vector.memset(V[i][:, HD:HD + 1], 1.0)
                    masks = attp.tile([128, 8, 512], BF)
                    for i in range(8):
                        delta = 512 - 128 * i
                        mk = masks[:, i, :]
                        nc.gpsimd.memset(mk, 1.0)
                        nc.gpsimd.affine_select(
                            out=mk, in_=mk, compare_op=ALU.is_ge, fill=0.0,
                            base=delta, pattern=[[1, 512]], channel_multiplier=-1)
                        nc.gpsimd.affine_select(
                            out=mk, in_=mk, compare_op=ALU.is_ge, fill=0.0,
                            base=-delta + (WIN - 1), pattern=[[-1, 512]],
                            channel_multiplier=1)
                    wo_sb = attp.tile([128, 2, HID], BF, name="wo_sb")
                    nc.sync.dma_start(wo_sb[:],
                                      wo_w.ap().rearrange("(a p) n -> p a n", p=128))

                    with tc.tile_pool(name="s2", bufs=2) as s2:
                        for tt in range(S // 128):
                            jj, hh = (tt, 0) if tt < 8 else (tt - 8, 1)
                            base = 256 * jj + 128 * hh
                            rows = slice(base, base + 128)
                            qk_t = s2.tile([128, 2 * HD], BF, tag="qkt",
                                           name=f"qkt{tt}", bufs=3)
                            nc.sync.dma_start(qk_t[:, 0:HD], a2aq_out[rows, :])
                            nc.sync.dma_start(qk_t[:, HD:2 * HD],
                                              a2akv_out[rows, 0:HD])
                            nc.sync.dma_start(V[tt][:, 0:HD],
                                              a2akv_out[rows, HD:2 * HD])
                            for (qo, tbi, QKT, nm) in (
                                    (0, 0, QTm, "q"),
                                    (HD, 2, KTm, "k")):
                                src = qk_t[:, qo:qo + HD]
                                rinv = rmsnorm_rinv(src, HD, f"{nm}n{tt}")
                                ct = ropes[:, tbi, tt, :]
                                st_ = ropes[:, tbi + 1, tt, :]
                                t1 = s2.tile([128, HD], BF, tag="t1",
                                             name=f"t1{nm}{tt}", bufs=3)
                                t2 = s2.tile([128, HD], BF, tag="t2",
                                             name=f"t2{nm}{tt}", bufs=3)
                                nc.vector.scalar_tensor_tensor(
                                    t1[:], src, rinv[:], ct,
                                    op0=ALU.mult, op1=ALU.mult)
                                nc.gpsimd.tensor_mul(
                                    t2[:].rearrange("p (a b) -> p a b", a=2),
                                    _swap_ap(src),
                                    st_.rearrange("p (a b) -> p a b", a=2))
                                qr = s2.tile([128, HD], BF, tag="qr",
                                             name=f"qr{nm}{tt}", bufs=3)
                                nc.vector.scalar_tensor_tensor(
                                    qr[:], t2[:], rinv[:], t1[:],
                                    op0=ALU.mult, op1=ALU.add)
                                ptr = psP.tile([128, HD], BF, tag="tr",
                                               bufs=2, name=f"s2t{nm}{tt}")
                                for h in range(2):
                                    nc.tensor.transpose(
                                        ptr[:, h * 128:(h + 1) * 128],
                                        qr[:, h * 128:(h + 1) * 128], ident[:])
                                nc.vector.tensor_copy(
                                    QKT[:, :, 128 * tt:128 * (tt + 1)],
                                    ptr[:].rearrange("p (a b) -> p a b", a=2))

                    mark('S3')
                    with tc.tile_pool(name="s3", bufs=2) as s3:
                        w1_pa_b = s3.tile([128, HID], BF, bufs=1, name="w1pab")
                        w1_pf_b = s3.tile([128, HID], BF, bufs=1, name="w1pfb")
                        _bcast_row(nc, w1_pa_b, w1_pa, HID)
                        _bcast_row(nc, w1_pf_b, w1_pf, HID)
                        h2Ts = s3.tile([128, KH, 2 * TS], F8, bufs=1, name="h2Ts")

                        def s4_half(t, rs_out_t):
                            ao = s3.tile([128, HID], BF, tag="ao", name=f"ao{t}",
                                         bufs=2)
                            nc.sync.dma_start(ao[:], rs_out_t[:])
                            rinv_a = rmsnorm_rinv(ao[:], HID, f"pan{t}")
                            x2 = s3.tile([128, HID], F32, tag="x2",
                                         name=f"x2_{t}", bufs=1)
                            for cch in range(5):
                                sl = slice(cch * 512, (cch + 1) * 512)
                                nc.vector.scalar_tensor_tensor(
                                    x2[:, sl], ao[:, sl], rinv_a[:],
                                    w1_pa_b[:, sl], op0=ALU.mult, op1=ALU.mult)
                                nc.gpsimd.tensor_add(x2[:, sl], x2[:, sl],
                                                      x_sb[t][:, sl])
                            nc.sync.dma_start(
                                x2_spill[t * 128:(t + 1) * 128, :], x2[:])
                            rinv_f = rmsnorm_rinv(x2[:], HID, f"pff{t}")
                            h2 = s3.tile([128, HID], BF, tag="h2", name=f"h2_{t}",
                                         bufs=2)
                            for cch in range(5):
                                sl = slice(cch * 512, (cch + 1) * 512)
                                nc.vector.scalar_tensor_tensor(
                                    h2[:, sl], x2[:, sl], rinv_f[:],
                                    w1_pf_b[:, sl], op0=ALU.mult, op1=ALU.mult)
                            for k in range(KH):
                                ptr = psP.tile([128, 128], BF, tag="tr", bufs=2,
                                               name=f"s4tr{k}_{t}")
                                nc.tensor.transpose(
                                    ptr[:], h2[:, k * 128:(k + 1) * 128],
                                    ident[:])
                                tsl = slice(t * 128, (t + 1) * 128)
                                lsl = slice(TS + t * 128, TS + (t + 1) * 128)
                                nc.vector.tensor_copy(h2Ts[:, k, tsl], ptr[:])
                                nc.vector.tensor_sub(h2Ts[:, k, lsl], ptr[:],
                                                     h2Ts[:, k, tsl])

                        for qb in range(4):
                            q0 = 512 * qb
                            probs = {}
                            for i in range(8):
                                kc = q0 - 512 + 128 * i
                                if kc < 0:
                                    continue
                                psc = psP.tile([128, 512], F32, tag="mm", bufs=6,
                                               name=f"psc{qb}_{i}")
                                for h in range(2):
                                    nc.tensor.matmul(psc[:], KTm[:, h, kc:kc + 128],
                                                     QTm[:, h, q0:q0 + 512],
                                                     start=(h == 0), stop=(h == 1))
                                pr = s3.tile([128, 512], BF, tag="pr",
                                             name=f"pr{qb}_{i}", bufs=10)
                                nc.scalar.activation(pr[:], psc[:], AF.Exp,
                                                     scale=1.0 / 16.0)
                                nc.vector.tensor_mul(pr[:], pr[:], masks[:, i, :])
                                probs[kc] = pr
                            for qs in range(4):
                                qa = q0 + 128 * qs
                                kcs = [kc for kc in range(qa - 512, qa + 128, 128)
                                       if kc >= 0]
                                po = psP.tile([128, HD + 1], F32, tag="mm", bufs=6,
                                              name=f"po{qb}_{qs}")
                                col = qa - q0
                                for j, kc in enumerate(kcs):
                                    nc.tensor.matmul(po[:],
                                                     probs[kc][:, col:col + 128],
                                                     V[kc // 128][:], start=(j == 0),
                                                     stop=(j == len(kcs) - 1))
                                rec = s3.tile([128, 1], F32, tag="rec",
                                              name=f"rec{qb}_{qs}")
                                nc.vector.reciprocal(rec[:], po[:, HD:HD + 1])
                                an = s3.tile([128, HD], BF, tag="an",
                                             name=f"an{qb}_{qs}")
                                nc.vector.tensor_scalar_mul(an[:], po[:, 0:HD],
                                                            rec[:])
                                ptr = psP.tile([128, HD], BF, tag="tr", bufs=2,
                                               name=f"s3tr{qb}{qs}")
                                for h in range(2):
                                    nc.tensor.transpose(
                                        ptr[:, h * 128:(h + 1) * 128],
                                        an[:, h * 128:(h + 1) * 128], ident[:])
                                aT = s3.tile([128, 2, 128], BF, tag="aTh",
                                             name=f"aTh{qb}_{qs}", bufs=3)
                                p3 = ptr[:].rearrange("p (a b) -> p a b", a=2)
                                nc.vector.tensor_copy(aT[:], p3)
                                # row-parallel wo partials for this token tile
                                xo = s3.tile([128, HID], BF, tag="xo",
                                             name=f"xo{qb}_{qs}", bufs=3)
                                for n in range(5):
                                    pw = psP.tile([128, 512], F32, tag="mm",
                                                  bufs=6, name=f"pw{qb}{qs}_{n}")
                                    nsl = slice(n * 512, (n + 1) * 512)
                                    for h in range(2):
                                        nc.tensor.matmul(pw[:], aT[:, h, :],
                                                         wo_sb[:, h, nsl],
                                                         start=(h == 0),
                                                         stop=(h == 1))
                                    if n % 2 == 0:
                                        nc.scalar.activation(xo[:, nsl], pw[:],
                                                             AF.Copy)
                                    else:
                                        nc.vector.tensor_copy(xo[:, nsl], pw[:])
                                qq = qa // 128
                                if qq < 8:
                                    nc.sync.dma_start(
                                        rs1a_in[128 * qq:128 * (qq + 1), :], xo[:])
                                else:
                                    nc.sync.dma_start(
                                        rs1b_in[128 * (qq - 8):128 * (qq - 7), :],
                                        xo[:])
                            if qb == 1:
                                _coll("ReduceScatter", ALU.add, [rs1a_in[:]],
                                      [rs1a_out[:]])
                                s4_half(0, rs1a_out)
                            elif qb == 3:
                                _coll("ReduceScatter", ALU.add, [rs1b_in[:]],
                                      [rs1b_out[:]])
                                s4_half(1, rs1b_out)
                        nc.sync.dma_start(
                            h2T_in[:].rearrange("(k p) t -> p k t", p=128),
                            h2Ts[:])
                        if not sim:
                            _coll("AllGather", ALU.bypass, [h2T_in[:]],
                                  [h2T_full[:]])

            mark('S5')
            # ================= S5: MLP (fp8 split3) =================
            with tc.tile_pool(name="s5w", bufs=1) as s5w:
                h2T_sb = s5w.tile([128, KH, NC_, 2 * TS], F8, name="h2T_sb")
                actT_hi = s5w.tile([128, MI, S], F8, name="actT_hi")
                actT_lo = s5w.tile([128, MI, S], F8, name="actT_lo")
                with tc.tile_pool(name="s5", bufs=2) as s5:
                    def load_wu(m):
                        d = {}
                        for (sfx, t_) in (("h", wu_hi), ("l", wu_lo),
                                          ("gl", wg_lo)):
                            w_ = s5.tile([128, KH, 128], F8, tag=f"wu{sfx}",
                                         name=f"wu{sfx}{m}", bufs=2)
                            nc.sync.dma_start(w_[:], t_.ap()[m])
                            d[sfx] = w_
                        return d

                    wu_tiles = {0: load_wu(0)}
                    for r in range(NC_):
                        blk = h2T_full[r * HID:(r + 1) * HID]
                        if sim:
                            nc.sync.dma_start(blk, h2T_in[:])
                        nc.sync.dma_start(
                            h2T_sb[:, :, r, :],
                            blk.rearrange("(k p) t -> p k t", p=128))
                        if r == 1:
                            wu_tiles[1] = load_wu(1)
                    for m in range(MI):
                        msl = slice(m * 128, (m + 1) * 128)
                        wu_t = wu_tiles.pop(m)
                        if m + 2 < MI:
                            wu_tiles[m + 2] = load_wu(m + 2)
                        for r in range(0, NC_, 2):
                            pg = psP.tile([128, 512], F32, tag="mm", bufs=6,
                                          name=f"pg{m}_{r}")
                            pu = psP.tile([128, 512], F32, tag="mm", bufs=6,
                                          name=f"pu{m}_{r}")
                            for kp in range(KH // 2):
                                ks = slice(2 * kp, 2 * kp + 2)
                                ah = h2T_sb[:, ks, r:r + 2, 0:TS]
                                al = h2T_sb[:, ks, r:r + 2, TS:2 * TS]
                                st = (kp == 0)
                                sp = (kp == KH // 2 - 1)
                                for (pp, wh_ap, wl_ap) in (
                                        (pg, wg_pre_hi[:, ks, msl],
                                         wu_t["gl"][:, ks, :]),
                                        (pu, wu_t["h"][:, ks, :],
                                         wu_t["l"][:, ks, :])):
                                    nc.tensor.matmul(pp[:], wh_ap, ah,
                                                     start=st, stop=False,
                                                     perf_mode=DR)
                                    nc.tensor.matmul(pp[:], wl_ap, ah,
                                                     start=False, stop=False,
                                                     perf_mode=DR)
                                    nc.tensor.matmul(pp[:], wh_ap, al,
                                                     start=False, stop=sp,
                                                     perf_mode=DR)
                            gsc = s5.tile([128, 512], F32, tag="gsc",
                                          name=f"gsc{m}_{r}", bufs=3)
                            nc.scalar.activation(gsc[:], pg[:],
                                                 AF.Gelu_apprx_tanh, scale=IWS)
                            tfull = s5.tile([128, 512], BF, tag="tfull",
                                            name=f"tf{m}_{r}", bufs=3)
                            nc.vector.scalar_tensor_tensor(
                                tfull[:], pu[:], IWS, gsc[:],
                                op0=ALU.mult, op1=ALU.mult)
                            csl = slice(r * TS, (r + 2) * TS)
                            nc.vector.tensor_copy(actT_hi[:, m, csl], tfull[:])
                            nc.gpsimd.tensor_sub(actT_lo[:, m, csl], tfull[:],
                                                 actT_hi[:, m, csl])
                    for n in range(5):
                        nsl = slice(n * 512, (n + 1) * 512)
                        wdn_hi = s5.tile([128, MI, 512], F8, tag="wdnh",
                                         name=f"wdnh{n}", bufs=2)
                        wdn_lo = s5.tile([128, MI, 512], F8, tag="wdnl",
                                         name=f"wdnl{n}", bufs=2)
                        nc.sync.dma_start(wdn_hi[:], wd_hi.ap()[:, nsl]
                                          .rearrange("(i p) n -> p i n", p=128))
                        nc.sync.dma_start(wdn_lo[:], wd_lo.ap()[:, nsl]
                                          .rearrange("(i p) n -> p i n", p=128))
                        for tt in range(S // 128):
                            csl = slice(128 * tt, 128 * (tt + 1))
                            pd = psP.tile([128, 512], F32, tag="mm", bufs=6,
                                          name=f"pd{n}_{tt}")
                            for ip in range(MI // 2):
                                isl = slice(2 * ip, 2 * ip + 2)
                                st = (ip == 0)
                                sp = (ip == MI // 2 - 1)
                                nc.tensor.matmul(pd[:], actT_hi[:, isl, csl],
                                                 wdn_hi[:, isl, :], start=st,
                                                 stop=False, perf_mode=DR)
                                nc.tensor.matmul(pd[:], actT_lo[:, isl, csl],
                                                 wdn_hi[:, isl, :], start=False,
                                                 stop=False, perf_mode=DR)
                                nc.tensor.matmul(pd[:], actT_hi[:, isl, csl],
                                                 wdn_lo[:, isl, :], start=False,
                                                 stop=sp, perf_mode=DR)
                            dcp = s5.tile([128, 512], BF, tag="dcp",
                                          name=f"dcp{n}_{tt}", bufs=4)
                            if tt % 2 == 0:
                                nc.vector.tensor_scalar_mul(dcp[:], pd[:], IWS)
                            else:
                                nc.scalar.activation(dcp[:], pd[:], AF.Copy,
                                                     scale=IWS)
                            nc.sync.dma_start(rs2_in[csl, nsl], dcp[:])
                    _coll("ReduceScatter", ALU.add, [rs2_in[:]], [rs2_out[:]])
            wgpre_cm.__exit__(None, None, None)

            mark('S6')
            # ============ S6: post_ff norm + residual ============
            with tc.tile_pool(name="s6", bufs=2) as s6:
                w1_po_b = s6.tile([128, HID], F32, bufs=1)
                _bcast_row(nc, w1_po_b, w1_po, HID)
                for t in range(2):
                    rows = slice(t * 128, (t + 1) * 128)
                    mlp16 = s6.tile([128, HID], BF, tag="mlp", name=f"mlp{t}",
                                    bufs=2)
                    nc.sync.dma_start(mlp16[:], rs2_out[rows, :])
                    x2l = s6.tile([128, HID], F32, tag="x2l", name=f"x2l{t}",
                                  bufs=2)
                    nc.sync.dma_start(x2l[:], x2_spill[rows, :])
                    rinv_o = rmsnorm_rinv(mlp16[:], HID, f"pon{t}")
                    o32 = s6.tile([128, HID], F32, tag="o32", name=f"o32_{t}",
                                  bufs=2)
                    nc.vector.scalar_tensor_tensor(o32[:], mlp16[:], rinv_o[:],
                                                   w1_po_b[:], op0=ALU.mult,
                                                   op1=ALU.mult)
                    nc.vector.tensor_add(o32[:], o32[:], x2l[:])
                    nc.sync.dma_start(out_shard.ap()[rows, :], o32[:])

    nc.compile()
    return nc


_NC_CACHE = None


def _get_nc():
    global _NC_CACHE
    if _NC_CACHE is None:
        _NC_CACHE = build_nc()
    return _NC_CACHE


def _split8(w, scale):
    """fp8 e4m3 hi/lo split of w*scale (hi+lo ~= w*scale to ~fp8^2)."""
    e4m3 = ml_dtypes.float8_e4m3
    ws = np.asarray(w, np.float32) * scale
    hi = ws.astype(e4m3)
    lo = (ws - hi.astype(np.float32)).astype(e4m3)
    return np.ascontiguousarray(hi), np.ascontiguousarray(lo)


def make_in_maps(hidden_states, position_ids, wq, wk, wv, wo, q_ln_w, k_ln_w,
                 in_ln_w, post_attn_ln_w, pre_ff_ln_w, post_ff_ln_w,
                 w_gate, w_up, w_down):
    bf16 = ml_dtypes.bfloat16
    f32 = np.float32
    x = np.asarray(hidden_states, f32).reshape(S, HID)
    pos = np.asarray(position_ids).reshape(S).astype(np.float64)

    inv_freq = 1.0 / (BASE ** (np.arange(0, HD, 2, dtype=np.float64) / HD))
    freqs = pos[:, None] * inv_freq[None, :]
    emb = np.concatenate([freqs, freqs], axis=1)
    cos = np.cos(emb).astype(f32)
    sin = np.sin(emb).astype(f32)
    w1q = 1.0 + np.asarray(q_ln_w, f32)
    w1k = 1.0 + np.asarray(k_ln_w, f32)

    def rope_tabs(w1):
        w1sw = np.concatenate([w1[HALF:], w1[:HALF]])
        sgn = np.concatenate([-np.ones(HALF, f32), np.ones(HALF, f32)])
        return ((cos * w1[None, :]).astype(bf16),
                (sin * (w1sw * sgn)[None, :]).astype(bf16))

    cqw_np, sqw_np = rope_tabs(w1q)
    ckw_np, skw_np = rope_tabs(w1k)

    wqkv = np.concatenate([np.asarray(wq, f32), np.asarray(wk, f32),
                           np.asarray(wv, f32)], axis=1)  # [HID, 4096]
    wqkv_hi, wqkv_lo = _split8(wqkv, WS)
    bf16_t = ml_dtypes.bfloat16
    wo_r = np.asarray(wo, f32).reshape(NH, HD, HID)
    wg_r = np.asarray(w_gate, f32).reshape(HID, NC_, INTER // NC_)
    wu_r = np.asarray(w_up, f32).reshape(HID, NC_, INTER // NC_)
    wd_r = np.asarray(w_down, f32).reshape(NC_, INTER // NC_, HID)

    def _pmajor(w2d):
        # [HID, 1280] -> [MI, 128, KH, 128] (per m-tile, partition-major)
        w4 = np.asarray(w2d).reshape(KH, 128, MI, 128)
        return np.ascontiguousarray(w4.transpose(2, 1, 0, 3))

    common = {
        "wqkv_hi": wqkv_hi, "wqkv_lo": wqkv_lo,
        "w1_in": (1.0 + np.asarray(in_ln_w, f32)).astype(bf16),
        "w1_pa": (1.0 + np.asarray(post_attn_ln_w, f32)).astype(bf16),
        "w1_pf": (1.0 + np.asarray(pre_ff_ln_w, f32)).astype(bf16),
        "w1_po": 1.0 + np.asarray(post_ff_ln_w, f32),
        "cqw": cqw_np, "sqw": sqw_np, "ckw": ckw_np, "skw": skw_np,
    }
    in_maps = []
    for c in range(NC_):
        wg_hi_c, wg_lo_c = _split8(wg_r[:, c, :], WS)
        wu_hi_c, wu_lo_c = _split8(wu_r[:, c, :], WS)
        wg_lo_c = _pmajor(wg_lo_c)
        wu_hi_c = _pmajor(wu_hi_c)
        wu_lo_c = _pmajor(wu_lo_c)
        wd_hi_c, wd_lo_c = _split8(wd_r[c], WS)
        in_maps.append({
            "x_shard": np.ascontiguousarray(np.concatenate(
                [x[128 * c:128 * (c + 1)],
                 x[1024 + 128 * c:1024 + 128 * (c + 1)]], axis=0)),
            "wo_w": np.ascontiguousarray(wo_r[c]).astype(bf16_t),
            "wg_hi": wg_hi_c, "wg_lo": wg_lo_c,
            "wu_hi": wu_hi_c, "wu_lo": wu_lo_c,
            "wd_hi": wd_hi_c, "wd_lo": wd_lo_c,
            **common,
        })
    return in_maps


def kernel(**inputs):
    in_maps = make_in_maps(**inputs)
    nc = _get_nc()
    res = run_bass_kernel_spmd(nc, in_maps, core_ids=list(range(NC_)))
    out = np.empty((S, HID), np.float32)
    for c in range(NC_):
        sh = res.results[c]["out_shard"]
        out[128 * c:128 * (c + 1)] = sh[0:128]
        out[1024 + 128 * c:1024 + 128 * (c + 1)] = sh[128:256]
    return out.reshape(1, S, HID).astype(np.float32)
